# revision 47
# baseline (speedup 1.0000x reference)
"""Trainium2 Bass kernel for an attention-LSTM decoder (Bahdanau attention +
LSTM cell + generator head), data-parallel over 8 NeuronCores.

Shapes (hardcoded): B=1024, T=64, D=256, H=256, C=96, steps S=26.
Each core handles 128 batch rows.

Device layouts (per core, b = 128):
  - score chain runs "transposed": partitions = hidden dim tiles (2 x 128),
    free = (b, t) flat b-major.
  - softmax/context run natural: partitions = b, free = t / d.
  - gates + LSTM pointwise run TRANSPOSED (partitions = gate-dim tiles,
    free = b) so h lands directly in hidT layout - no h transpose.
Matmul operands are fp16 (full-rate PE streaming); PSUM accumulation is
fp32; the recurrent c state is fp16 (transposed).

The recurrence is independent per batch row, so each step is split into
two b-halves that pipeline through the engines: half A's gates -> LSTM ->
hp chain finishes first and its z(s+1) stream starts while half B is
still in its step-s tail. Only the alpha*H chain and context need full
width. Per-step pipeline (per half): z = projT + hp (DVE, bcast AP; bh2h
pre-folded into projT; hp drained from psum directly in 2x-replicated
form) -> tanh (ACT) -> e-matmul (M=128-replicated w_score) into 2-bank
psum waves -> drain fp16 -> DMA-scatter to [b, t]. The (96,128) tail
gates the whole next phase, so chunks 12-15 drain first, the last wave
borrows the misc psum slot (free during the z phase) and keeps drain +
DMA issue + sigmoid on one ACT FIFO. Softmax r = 1/sigmoid(-e) = alpha+1
(resident sigmoid table, no exp switch, no normalize op): the context
psum is seeded with -sum_t H_t (fp16 residual pair) so unnormalized r
works directly, and 1/Z = 1/(sum r - T) is computed off-path. alpha*H
runs full-width split across DVE (4x tensor_scalar) / ACT (scale AP) /
Pool (one fused op), PE identity-accumulates into the seeded psum.
Gates + LSTM pointwise run per half in the transposed layout with one
fused sigmoid over i|f|o; gate psum is (half, mg, b')-major so each half
owns disjoint banks. Generator matmuls are emitted at the top of the
next step so the PE queue reaches them while stalled on alpha.

Host-side prep (numpy): fp16 casts, batch_H transpose for the projection
matmul, one-hot text encoding, gate columns permuted keras (i,f,c,o) ->
(i,f,o,c), b_lstm folded into the one-hot weight rows (valid because
one-hot rows sum to 1), bg added to the final output.
"""

import sys

for _p in ("/opt/trn_rl_repo", "/root/.axon_site/_ro/trn_rl_repo"):
    if _p not in sys.path:
        sys.path.insert(0, _p)

import numpy as np

import concourse.bass as bass
import concourse.tile as tile
from concourse import mybir
from concourse.bass_utils import run_bass_kernel_spmd
from concourse.masks import make_identity

dt = mybir.dt
AF = mybir.ActivationFunctionType
ALU = mybir.AluOpType

NCORES = 8
B, T, D, H, C = 1024, 64, 256, 256, 96
S = 26  # num steps = batch_max_length + 1
BC = B // NCORES  # 128 batch rows per core
KT = 2  # 256 = 2 x 128 tiles for d/h contraction
GT = 8  # 4H = 1024 = 8 m-tiles of 128
TB = BC * T  # 8192, flat (b, t) b-major
NCHUNK = 512  # psum-bank-limited matmul N
EWAVE = 2  # e-matmul psum slots are 2 banks (eB/eA2); the transposed
# gates psums multiplex onto eB (all uses sequential within a step),
# single-chunk waves ride the 1-bank "gen" slot, and the last 2-chunk
# wave borrows the misc slot (free during the z phase): 2+2+2+1 banks.
POOL_T = 8  # trailing t's of the alpha*H chain handled by one Pool op
# (Pool Multiply runs at 0.42 efficiency: 8 t's ~ 4.2us; Pool does no
# other per-step work since it cannot touch PSUM)

_CACHE = {}


def _split_excess_waits(nc, max_waits=1):
    """This container's walrus rejects instructions carrying more than
    ~max_waits semaphore waits ("Too many sync wait commands"). Hoist excess
    waits onto InstNoOp instructions inserted just before, on the same engine
    (per-engine program order makes this semantics-preserving)."""
    nid = [0]
    for f in nc.m.functions:
        for blk in f.blocks:
            insts = blk.instructions
            out = []
            changed = False
            for ins in insts:
                si = ins.sync_info
                ow = list(si.on_wait) if si is not None and si.on_wait else []
                if len(ow) > max_waits:
                    changed = True
                    while len(ow) > max_waits:
                        take, ow = ow[:max_waits], ow[max_waits:]
                        nid[0] += 1
                        nop = mybir.InstNoOp(
                            name=f"WSPLIT-{nid[0]}", engine=ins.engine,
                            sync_info=mybir.SyncInfo(on_wait=take,
                                                     on_update=[]))
                        nc.register_instruction(nop, overwrite=True)
                        out.append(nop)
                    ins.sync_info = mybir.SyncInfo(
                        on_wait=ow, on_update=list(si.on_update or []))
                out.append(ins)
            if changed:
                blk.instructions = out


def _build():
    nc = bass.Bass("TRN2", target_bir_lowering=False)
    f16, f32 = dt.float16, dt.float32

    h_nat_d = nc.declare_dram_parameter("h_nat", [BC, T, D], f16, isOutput=False)
    h_t_d = nc.declare_dram_parameter("h_t", [D, BC, T], f16, isOutput=False)
    oneh_d = nc.declare_dram_parameter("onehot", [C, S, BC], f16, isOutput=False)
    wi2h_d = nc.declare_dram_parameter("wi2h", [D, H], f16, isOutput=False)
    wh2h_d = nc.declare_dram_parameter("wh2h", [H, H], f16, isOutput=False)
    bh2h_d = nc.declare_dram_parameter("bh2hT", [H, 1], f32, isOutput=False)
    wsc_d = nc.declare_dram_parameter("wsc", [H, 128], f16, isOutput=False)
    wxc_d = nc.declare_dram_parameter("wxc", [D, 4 * H], f16, isOutput=False)
    wxo_d = nc.declare_dram_parameter("wxo", [C, 4 * H], f16, isOutput=False)
    wh_d = nc.declare_dram_parameter("wh", [H, 4 * H], f16, isOutput=False)
    wg_d = nc.declare_dram_parameter("wg", [H, C], f16, isOutput=False)
    probs_d = nc.declare_dram_parameter("probsT", [C, S, BC], f32, isOutput=True)

    with tile.TileContext(nc) as tc:
        import contextlib
        ctx = contextlib.ExitStack()
        with ctx:
            singles = ctx.enter_context(tc.tile_pool(name="singles", bufs=1))
            # psA: two 2-bank e-matmul wave slots (eB/eA2) so the PE can
            # fill one while the other drains; the transposed gates psums
            # multiplex onto eB (all uses sequential within a step).
            psA = ctx.enter_context(tc.tile_pool(name="psA", bufs=1, space="PSUM"))
            psB = ctx.enter_context(tc.tile_pool(name="psB", bufs=1, space="PSUM"))

            # ---- persistent SBUF state ----
            h_nat = singles.tile([BC, T, D], f16)
            projT = singles.tile([128, KT, BC, T], f16)  # [h', m, b, t]
            hidT = singles.tile([128, KT, S + 1, BC], f16)  # h states, slot 0 = 0
            oneh = singles.tile([C, S, BC], f16)
            wi2h = singles.tile([128, KT, H], f16)
            wh2h = singles.tile([128, KT, H], f16)
            bh2hT = singles.tile([128, KT, 1], f32)
            wsc = singles.tile([128, KT, 128], f16)
            wxc = singles.tile([128, KT, 4 * H], f16)
            wxo = singles.tile([C, 4 * H], f16)
            wh = singles.tile([128, KT, 4 * H], f16)
            wg = singles.tile([128, KT, C], f16)
            ident = singles.tile([128, 128], f16)
            cT = singles.tile([128, KT, BC], f16)
            probs_sb = singles.tile([C, S, BC], f32)

            # ---- load everything ----
            # (h_nat is issued after the h_tt chunks below: it is not read
            # until step 0's context phase, but 4 MB at the front of the
            # sync queue would delay the proj-critical h_tt transfer)
            nc.sync.dma_start(out=oneh, in_=oneh_d[:])
            nc.sync.dma_start(
                out=wi2h, in_=wi2h_d[:].rearrange("(k p) h -> p k h", p=128))
            nc.sync.dma_start(
                out=wh2h, in_=wh2h_d[:].rearrange("(k p) h -> p k h", p=128))
            nc.sync.dma_start(
                out=bh2hT, in_=bh2h_d[:].rearrange("(k p) o -> p k o", p=128))
            nc.sync.dma_start(
                out=wsc, in_=wsc_d[:].rearrange("(k p) o -> p k o", p=128))
            nc.sync.dma_start(
                out=wxc, in_=wxc_d[:].rearrange("(k p) g -> p k g", p=128))
            nc.sync.dma_start(out=wxo, in_=wxo_d[:])
            nc.sync.dma_start(
                out=wh, in_=wh_d[:].rearrange("(k p) g -> p k g", p=128))
            nc.sync.dma_start(
                out=wg, in_=wg_d[:].rearrange("(k p) c -> p k c", p=128))
            make_identity(nc, ident)
            nc.vector.memset(hidT[:, :, 0, :], 0.0)
            nc.vector.memset(cT, 0.0)

            # ---- precompute projT = (batch_H @ Wi2h)^T + bh2h : [h',m,(b t)]
            projT_f = projT[:].rearrange("p m b t -> p m (b t)")
            with tc.tile_pool(name="ht", bufs=1) as ht_pool:
                h_tt = ht_pool.tile([128, KT, BC, T], f16)
                # load in 4 b-chunks so the first proj matmuls start after a
                # quarter of the transfer instead of all of it
                for lb in range(4):
                    bls = slice(lb * (BC // 4), (lb + 1) * (BC // 4))
                    nc.sync.dma_start(
                        out=h_tt[:, :, bls, :],
                        in_=h_t_d[:, bls, :].rearrange(
                            "(k p) b t -> p k b t", p=128))
                nc.sync.dma_start(out=h_nat, in_=h_nat_d[:])
                h_tt_f = h_tt[:].rearrange("p k b t -> p k (b t)")
                nchk = TB // NCHUNK
                ncw = (nchk + EWAVE - 1) // EWAVE
                for m in range(KT):
                    for w in range(ncw):
                        nb = min(EWAVE, nchk - w * EWAVE)
                        ps = psA.tile([128, EWAVE, NCHUNK], f32,
                                      tag=("eB" if w % 2 == 0 else "eA2"))
                        for j in range(nb):
                            sl = slice((w * EWAVE + j) * NCHUNK,
                                       (w * EWAVE + j + 1) * NCHUNK)
                            for k in range(KT):
                                nc.tensor.matmul(
                                    ps[:, j, :],
                                    wi2h[:, k, m * 128:(m + 1) * 128],
                                    h_tt_f[:, k, sl], start=(k == 0),
                                    stop=(k == KT - 1))
                        sl3 = slice(w * EWAVE * NCHUNK,
                                    (w * EWAVE + nb) * NCHUNK)
                        pin = ps[:, :nb, :].rearrange("p a n -> p (a n)")
                        # fold the bh2h bias in during the drain, DVE/ACT
                        # alternating (Identity, unlike Copy, takes an AP
                        # bias; Pool can't read psum) - preamble only
                        if w % 2 == 0:
                            nc.vector.tensor_scalar(
                                out=projT_f[:, m, sl3], in0=pin,
                                scalar1=bh2hT[:, m, :], scalar2=None,
                                op0=ALU.add)
                        else:
                            nc.scalar.activation(
                                out=projT_f[:, m, sl3], in_=pin,
                                func=AF.Identity, bias=bh2hT[:, m, :])

            work = ctx.enter_context(tc.tile_pool(name="work", bufs=2))
            small = ctx.enter_context(tc.tile_pool(name="small", bufs=2))
            ahp = ctx.enter_context(tc.tile_pool(name="ahp", bufs=16))

            # ---- Hsum = sum_t H[:, t, :] (for the unnormalized-alpha
            # context trick: sum_t r_t H_t - Hsum = sum_t alpha_t H_t with
            # r = 1/sig(-e) = alpha + 1). Stored negated as an fp16
            # residual pair so the psum pre-load is fp32-exact.
            negA = singles.tile([BC, D], f16)
            negB = singles.tile([BC, D], f16)
            # dedicated drain buffer for the final e chunk: the rolling esb
            # pool would serialize the tail behind older waves' drains/DMAs
            esb15 = singles.tile([128, NCHUNK], f16)
            ps_hs = psB.tile([BC, D], dt.float32, tag="misc")
            for t in range(T):
                nc.tensor.matmul(ps_hs, ident, h_nat[:, t, :],
                                 start=(t == 0), stop=(t == T - 1))
            nc.scalar.activation(out=negA, in_=ps_hs, func=AF.Copy,
                                 scale=-1.0)
            hs32 = singles.tile([BC, D], dt.float32)
            nc.vector.tensor_scalar(out=hs32, in0=ps_hs, scalar1=-1.0,
                                    scalar2=None, op0=ALU.mult)
            nc.vector.tensor_tensor(out=negB, in0=hs32, in1=negA,
                                    op=ALU.subtract)

            # ---- decode steps ----
            def emit_gen(s):
                # generator matmuls for step s (reads hidT slot s+1); emitted
                # at the top of step s+1 so the PE queue reaches them while
                # stalled on step s+1's alpha (fills the tail gap). The
                # per-step output DMA hides the 1.3 MB store under compute.
                ps_p = psB.tile([C, 128], f32, tag="gen")
                for k in range(KT):
                    nc.tensor.matmul(
                        ps_p, wg[:, k, :], hidT[:, k, s + 1, :],
                        start=(k == 0), stop=(k == KT - 1))
                eng = nc.scalar if (s % 2 == 0) else nc.vector
                if eng is nc.scalar:
                    eng.copy(out=probs_sb[:, s, :], in_=ps_p)
                else:
                    eng.tensor_copy(probs_sb[:, s, :], ps_p)
                nc.sync.dma_start(out=probs_d[:, s, :], in_=probs_sb[:, s, :])

            # ---- half-pipelined decode ----
            # The recurrence is independent per batch row, and in both the
            # score layout (partitions = h') and the transposed gate/LSTM
            # layout (partitions = g') the batch lives on the free axis, so
            # splitting b into halves halves those ops' cost. Half A's
            # gates->LSTM->hp chain finishes first and its z(s+1) stream
            # starts while half B is still in its step-s tail; only the
            # full-width phases (alpha*H chain, ctx) need both halves.
            esbT = [singles.tile([128, 2, NCHUNK], f16, name=f"esbT{i}")
                    for i in range(4)]
            hpRs = {}

            def emit_hp_half(s, hi):
                # hp = h @ Wh2h for one b-half, transposed [h', m, b'];
                # drained straight into the 2x-replicated form the z-add's
                # broadcast AP wants (last dim step-1)
                hsl = slice(hi * 64, hi * 64 + 64)
                ps_hp = psB.tile([128, KT, 64], f32, tag="misc")
                for m in range(KT):
                    for k in range(KT):
                        nc.tensor.matmul(
                            ps_hp[:, m, :],
                            wh2h[:, k, m * 128:(m + 1) * 128],
                            hidT[:, k, s, hsl], start=(k == 0),
                            stop=(k == KT - 1))
                hpR = small.tile([128, KT, 64, 2], f16, tag=f"hpR{hi}")
                for m in range(KT):
                    base = ps_hp[:, m, :]
                    nc.vector.tensor_copy(
                        hpR[:, m, :, :],
                        bass.AP(tensor=base.tensor, offset=base.offset,
                                ap=[base.ap[0], [base.ap[-1][0], 64],
                                    [0, 2]]))
                hpRs[hi] = hpR

            for hi in (0, 1):
                emit_hp_half(0, hi)

            for s in range(S):
                if s > 0:
                    emit_gen(s - 1)

                alpha_e = small.tile([BC, T], f16, tag="alphae")
                sden = small.tile([BC, T], f16, tag="sden")
                rsd = small.tile([BC, T], f32, tag="rsd")

                def emit_softmax(h0, h1):
                    # r = 1/sigmoid(-e) = exp(e) + 1: unnormalized alpha
                    # offset by +1; the context psum pre-subtracts Hsum to
                    # compensate. sig(-e) is exact via the resident sigmoid
                    # table; no exp table switch.
                    nc.scalar.activation(out=sden[h0:h1, :],
                                         in_=alpha_e[h0:h1, :],
                                         func=AF.Sigmoid, scale=-1.0)
                    with nc.allow_low_precision(
                            reason="sig(-e) in [0.05, 0.95]; fp16 adds "
                                   "~5e-4 rel to alpha, under the 2e-2 "
                                   "budget"):
                        nc.vector.reciprocal(out=rsd[h0:h1, :],
                                             in_=sden[h0:h1, :])

                # z = projT + hp (bcast over t) per half; tanh on ACT;
                # e = w . tanh per 512-col chunk into psum waves, drained
                # fp16 and DMA-scattered to [b, t]. Single-chunk waves ride
                # dedicated buffers + the 1-bank gen slot so the per-half
                # tails never wait on rolling resources.
                ths = {}

                def z_groups(hi, groups, b0, upto=None):
                    base_b = hi * 64
                    hpR = hpRs[hi]
                    first = True
                    for nbz in groups:
                        bsl = slice(b0, b0 + nbz)
                        for m in range(KT):
                            # half B's first-group m0 add rides the idle
                            # Pool engine: its tanh slot comes after the
                            # whole tanh-A stream, so Pool's 3.3x-slower
                            # add is fully hidden. It gets its own buffer
                            # so its long hold doesn't stall the z ring.
                            pool_z = (hi == 1 and m == 0
                                      and nbz in (32, 24))
                            z = work.tile([128, 32, T], f16,
                                          tag="zp" if pool_z else "z")
                            hb = hpR[:, m, b0 - base_b:, :]
                            eng = nc.gpsimd if pool_z else nc.vector
                            eng.tensor_tensor(
                                out=z[:, :nbz, :].rearrange(
                                    "p b (r i) -> p b r i", i=2),
                                in0=projT[:, m, bsl, :].rearrange(
                                    "p b (r i) -> p b r i", i=2),
                                in1=bass.AP(
                                    tensor=hb.tensor, offset=hb.offset,
                                    ap=[hb.ap[0], [hb.ap[1][0], nbz],
                                        [0, T // 2], [1, 2]]),
                                op=ALU.add)
                            first = False
                            th = work.tile([128, 32 * T], f16,
                                           tag=f"th{hi}{m}")
                            nc.scalar.activation(
                                out=th[:, :nbz * T],
                                in_=z[:, :nbz, :].rearrange(
                                    "p b t -> p (b t)"),
                                func=AF.Tanh)
                            for q in range(nbz * T // NCHUNK):
                                c = (b0 - base_b) // 8 + q + hi * 8
                                ent = ths.get(c, [None, None, q * NCHUNK])
                                ent[m] = th
                                ent[2] = q * NCHUNK
                                ths[c] = ent
                        b0 += nbz
                    return b0

                def e_mms(chunks, tag):
                    nb = len(chunks)
                    if tag == "gen":
                        ps_e = psB.tile([128, 1, NCHUNK], f32, tag="gen")
                    elif tag == "misc":
                        ps_e = psB.tile([128, 2, NCHUNK], f32, tag="misc")
                    else:
                        ps_e = psA.tile([128, EWAVE, NCHUNK], f32, tag=tag)
                    for j, c in enumerate(chunks):
                        th0, th1, off = ths[c]
                        thm = (th0, th1)
                        for m in range(KT):
                            nc.tensor.matmul(
                                ps_e[:, j, :], wsc[:, m, :],
                                thm[m][:, off:off + NCHUNK],
                                start=(m == 0), stop=(m == KT - 1))
                    return ps_e

                def e_out(chunks, ps_e, ded, deng, dma_eng=None):
                    nb = len(chunks)
                    if ded is not None:
                        ebuf = esbT[ded]
                        tgt = ebuf[:, :nb, :]
                    else:
                        ebuf = work.tile([128, EWAVE, NCHUNK], f16,
                                         tag="esb")
                        tgt = ebuf[:, :nb, :]
                    if deng == "act":
                        nc.scalar.copy(out=tgt, in_=ps_e[:, :nb, :])
                    else:
                        nc.vector.tensor_copy(tgt, ps_e[:, :nb, :])
                    eb = ebuf[0:1, 0:nb, :]
                    (dma_eng or nc.sync).dma_start(
                        out=alpha_e[chunks[0] * 8:
                                    chunks[0] * 8 + nb * 8, :],
                        in_=bass.AP(tensor=eb.tensor, offset=eb.offset,
                                    ap=[eb.ap[0], [1, nb * NCHUNK]]),
                        single_packet=True)

                # --- half A: z stream + waves, inline drains (its tail is
                # hidden under half B's z stream) ---
                b0 = z_groups(0, (8, 24, 32), 0)
                assert b0 == 64
                for chunks, tag, ded, deng in (
                        ((0,), "gen", 0, "dve"),
                        ((1, 2), "eB", None, "dve"),
                        ((3, 4), "eA2", None, "dve"),
                        ((5, 6), "eB", None, "dve"),
                        ((7,), "gen", 1, "act")):
                    ps_e = e_mms(chunks, tag)
                    e_out(chunks, ps_e, ded, deng)
                emit_softmax(0, 64)

                # --- half B: the (96,128) tail gates the whole ah phase,
                # so chunks 12-15 drain first; the last wave rides the
                # misc psum slot (free during the z phase) and its drain +
                # DMA issue + sigmoid all stay on the ACT FIFO ---
                b0 = z_groups(1, (32, 24), 64)
                ps8 = e_mms((8,), "gen")
                e_out((8,), ps8, 2, "dve")
                ps910 = e_mms((9, 10), "eA2")
                ps11 = e_mms((11,), "gen")
                b0 = z_groups(1, (8,), b0)
                assert b0 == BC
                ps1213 = e_mms((12, 13), "eB")
                ps1415 = e_mms((14, 15), "misc")
                e_out((12, 13), ps1213, None, "dve")
                e_out((14, 15), ps1415, 3, "act", dma_eng=nc.scalar)
                emit_softmax(96, BC)
                e_out((9, 10), ps910, None, "dve")
                e_out((11,), ps11, 0, "dve")
                emit_softmax(64, 96)

                # gates partial (h and onehot contributions don't need ctx);
                # transposed: out [g', b], full width. No data deps, so the
                # PE chews it during the softmax tail, keeping the pstate
                # ramp alive for the ident accumulates. Each m-tile's psum
                # group is closed immediately (one open accumulation group
                # per psum bank) and drained to SBUF; re-injected per half.
                ps_gp = psA.tile([128, GT, 128], f32, tag="eB")
                for mg in range(GT):
                    gsl = slice(mg * 128, (mg + 1) * 128)
                    for k in range(KT):
                        nc.tensor.matmul(
                            ps_gp[:, mg, :], wh[:, k, gsl],
                            hidT[:, k, s, :], start=(k == 0), stop=False)
                    nc.tensor.matmul(
                        ps_gp[:, mg, :], wxo[:, gsl], oneh[:, s, :],
                        start=False, stop=True)
                ghosb = work.tile([128, GT, 128], f16, tag="gho")
                nc.vector.tensor_copy(ghosb, ps_gp)

                # 1/Z for the later ctx rescale: Z = sum_t r - T, off the
                # critical path (only needed once the ah chain finishes)
                zs = small.tile([BC, 1], f32, tag="zsum")
                nc.vector.reduce_sum(zs, rsd, mybir.AxisListType.X)
                zm = small.tile([BC, 1], f32, tag="zm")
                nc.vector.tensor_scalar(out=zm, in0=zs, scalar1=-float(T),
                                        scalar2=None, op0=ALU.add)
                rzb = small.tile([BC, 1], f32, tag="rzb")
                nc.vector.reciprocal(out=rzb, in_=zm)

                # context = sum_t r[:, t] * H[:, t, :] - Hsum (r = alpha+1),
                # full width: r_t*H_t slices at 4x on DVE with ACT every 5th
                # t and Pool one fused op for the last POOL_T t's; PE
                # accumulates via identity matmuls into psum seeded with
                # -Hsum (fp16 residual pair, no data deps).
                ps_cx = psB.tile([BC, D], f32, tag="misc")
                nc.tensor.matmul(ps_cx, ident, negA, start=True, stop=False)
                nc.tensor.matmul(ps_cx, ident, negB, start=False, stop=False)
                TS = T - POOL_T
                ah_pool = work.tile([BC, POOL_T, D], f16, tag="ahpool")
                hb = h_nat[:, TS:, :]
                al = rsd[:, TS:]
                nc.gpsimd.tensor_tensor(
                    out=ah_pool, in0=hb,
                    in1=bass.AP(tensor=al.tensor, offset=al.offset,
                                ap=[al.ap[0], [al.ap[-1][0], POOL_T],
                                    [0, D]]),
                    op=ALU.mult)
                ahs = []
                for t in range(TS):
                    ah = ahp.tile([BC, D], f16, tag="ah")
                    if t % 5 == 4:
                        nc.scalar.activation(
                            out=ah, in_=h_nat[:, t, :], func=AF.Copy,
                            scale=rsd[:, t:t + 1])
                    else:
                        nc.vector.tensor_scalar(
                            out=ah, in0=h_nat[:, t, :],
                            scalar1=rsd[:, t:t + 1],
                            scalar2=None, op0=ALU.mult)
                    ahs.append(ah)
                    if t % 16 == 15 or t == TS - 1:
                        for a in ahs:
                            nc.tensor.matmul(ps_cx, ident, a, start=False,
                                             stop=False)
                        ahs = []
                assert not ahs
                for j in range(POOL_T):
                    nc.tensor.matmul(ps_cx, ident, ah_pool[:, j, :],
                                     start=False, stop=(j == POOL_T - 1))
                # 1/Z rescale on ACT (Copy takes a per-partition AP scale
                # and reads psum through the faster port)
                ctxv = small.tile([BC, D], f16, tag="ctxv")
                nc.scalar.activation(out=ctxv, in_=ps_cx, func=AF.Copy,
                                     scale=rzb[:, 0:1])
                # transpose context -> [d', m, b]
                ps_ct = psB.tile([BC, KT, 128], f16, tag="misc")
                ctxT = small.tile([128, KT, 128], f16, tag="ctxT")
                for m in range(KT):
                    nc.tensor.transpose(
                        ps_ct[:, m, :], ctxv[:, m * 128:(m + 1) * 128],
                        ident)
                    nc.vector.tensor_copy(ctxT[:, m, :], ps_ct[:, m, :])

                # per-half tail: gates -> LSTM -> hp. Emission order keeps
                # each engine FIFO clean: gatesA, gatesB (PE) / lstmA,
                # lstmB (ACT+DVE) / hpA, hpB (PE) with the hpR drains last,
                # so next step's half-A z stream starts while half B
                # finishes. Gate psum is (half, mg, b')-major: each half
                # owns disjoint banks, so B's accumulation never collides
                # with A's LSTM reads.
                ps_g = psA.tile([128, 2, GT, 64], f32, tag="eB")
                for hi in (0, 1):
                    hsl = slice(hi * 64, hi * 64 + 64)
                    for mg in range(GT):
                        gsl = slice(mg * 128, (mg + 1) * 128)
                        nc.tensor.matmul(
                            ps_g[:, hi, mg, :], ident, ghosb[:, mg, hsl],
                            start=True, stop=False)
                        for k in range(KT):
                            nc.tensor.matmul(
                                ps_g[:, hi, mg, :], wxc[:, k, gsl],
                                ctxT[:, k, hsl],
                                start=False, stop=(k == KT - 1))
                for hi in (0, 1):
                    hsl = slice(hi * 64, hi * 64 + 64)
                    # LSTM pointwise, transposed [g', b']. permuted gate
                    # order i,f,o,g -> m-tiles 0:2 = i, 2:4 = f, 4:6 = o,
                    # 6:8 = g (b_lstm folded into wxo host-side).
                    sig_ifo = small.tile([128, 6, 64], f16, tag="sig_ifo")
                    tg = small.tile([128, KT, 64], f16, tag="tg")
                    nc.scalar.activation(out=sig_ifo,
                                         in_=ps_g[:, hi, 0:6, :],
                                         func=AF.Sigmoid)
                    nc.scalar.activation(out=tg, in_=ps_g[:, hi, 6:8, :],
                                         func=AF.Tanh)
                    t1 = small.tile([128, KT, 64], f16, tag="t1")
                    t2 = small.tile([128, KT, 64], f16, tag="t2")
                    nc.vector.tensor_tensor(out=t1, in0=sig_ifo[:, 2:4, :],
                                            in1=cT[:, :, hsl], op=ALU.mult)
                    nc.vector.tensor_tensor(out=t2, in0=sig_ifo[:, 0:2, :],
                                            in1=tg, op=ALU.mult)
                    nc.vector.tensor_tensor(out=cT[:, :, hsl], in0=t1,
                                            in1=t2, op=ALU.add)
                    tc_t = small.tile([128, KT, 64], f16, tag="tc_t")
                    nc.scalar.activation(out=tc_t, in_=cT[:, :, hsl],
                                         func=AF.Tanh)
                    # h goes straight into hidT layout - no transpose
                    nc.vector.tensor_tensor(out=hidT[:, :, s + 1, hsl],
                                            in0=sig_ifo[:, 4:6, :],
                                            in1=tc_t, op=ALU.mult)
                    if s < S - 1:
                        emit_hp_half(s + 1, hi)

            emit_gen(S - 1)

    _split_excess_waits(nc)
    return nc


def _get_module():
    if "nc" not in _CACHE:
        _CACHE["nc"] = _build()
    return _CACHE["nc"]


def build_in_maps(batch_H, text, batch_max_length, Wi2h, Wh2h, bh2h, w_score,
                  Wx, Wh, b_lstm, Wg, bg):
    batch_H = np.asarray(batch_H, dtype=np.float32)
    text = np.asarray(text)
    assert int(batch_max_length) + 1 == S
    assert batch_H.shape == (B, T, D)

    f16 = np.float16
    bh16 = batch_H.astype(f16)
    # one-hot text: [B, S, C] -> per-core [S, C, BC]
    oh = (text[:, :S, None] == np.arange(C)[None, None, :])

    # permute keras gate order (i, f, c, o) -> (i, f, o, c)
    perm = np.concatenate([np.arange(0, 2 * H),          # i, f
                           np.arange(3 * H, 4 * H),      # o
                           np.arange(2 * H, 3 * H)])     # c/g
    Wx = np.asarray(Wx, np.float32)[:, perm]
    Wh_p = np.asarray(Wh, np.float32)[:, perm]
    bl_p = np.asarray(b_lstm, np.float32)[perm]
    wxo_p = (Wx[D:D + C, :] + bl_p[None, :]).astype(f16)
    weights = {
        "wi2h": np.ascontiguousarray(np.asarray(Wi2h, np.float32).astype(f16)),
        "wh2h": np.ascontiguousarray(np.asarray(Wh2h, np.float32).astype(f16)),
        "bh2hT": np.ascontiguousarray(
            np.asarray(bh2h, np.float32).reshape(H, 1)),
        "wsc": np.ascontiguousarray(np.tile(
            np.asarray(w_score, np.float32).reshape(H, 1), (1, 128)).astype(f16)),
        "wxc": np.ascontiguousarray(Wx[:D, :].astype(f16)),
        "wxo": np.ascontiguousarray(wxo_p),
        "wh": np.ascontiguousarray(Wh_p.astype(f16)),
        "wg": np.ascontiguousarray(np.asarray(Wg, np.float32).astype(f16)),
    }

    in_maps = []
    for c in range(NCORES):
        bsl = slice(c * BC, (c + 1) * BC)
        in_maps.append({
            "h_nat": np.ascontiguousarray(bh16[bsl]),
            "h_t": np.ascontiguousarray(bh16[bsl].transpose(2, 0, 1)),
            "onehot": np.ascontiguousarray(
                oh[bsl].transpose(2, 1, 0).astype(f16)),
            **weights,
        })
    return in_maps


def kernel(**inputs):
    in_maps = build_in_maps(**inputs)
    bg = inputs["bg"]

    nc = _get_module()
    res = run_bass_kernel_spmd(nc, in_maps, list(range(NCORES)))

    out = np.empty((B, S, C), np.float32)
    for c in range(NCORES):
        out[c * BC:(c + 1) * BC] = res.results[c]["probsT"].transpose(2, 1, 0)
    out += np.asarray(bg, np.float32)[None, None, :]
    return out


if __name__ == "__main__":
    _build()
    print("build OK")


# revision 49
# speedup vs baseline: 1.0393x; 1.0393x over previous
"""Trainium2 Bass kernel for an attention-LSTM decoder (Bahdanau attention +
LSTM cell + generator head), data-parallel over 8 NeuronCores.

Shapes (hardcoded): B=1024, T=64, D=256, H=256, C=96, steps S=26.
Each core handles 128 batch rows.

Device layouts (per core, b = 128):
  - score chain runs "transposed": partitions = hidden dim tiles (2 x 128),
    free = (b, t) flat b-major.
  - softmax/context run natural: partitions = b, free = t / d.
  - gates + LSTM pointwise run TRANSPOSED (partitions = gate-dim tiles,
    free = b) so h lands directly in hidT layout - no h transpose.
Matmul operands are fp16 (full-rate PE streaming); PSUM accumulation is
fp32; the recurrent c state is fp16 (transposed).

The recurrence is independent per batch row, so each step is split into
two b-halves that pipeline through the engines: half A's gates -> LSTM ->
hp chain finishes first and its z(s+1) stream starts while half B is
still in its step-s tail. Only the alpha*H chain and context need full
width. Per-step pipeline (per half): z = projT + hp (DVE, bcast AP; bh2h
pre-folded into projT; hp drained from psum directly in 2x-replicated
form) -> tanh (ACT) -> e-matmul (M=128-replicated w_score) into 2-bank
psum waves -> drain fp16 -> DMA-scatter to [b, t]. The (96,128) tail
gates the whole next phase, so chunks 12-15 drain first, the last wave
borrows the misc psum slot (free during the z phase) and keeps drain +
DMA issue + sigmoid on one ACT FIFO. Softmax r = 1/sigmoid(-e) = alpha+1
(resident sigmoid table, no exp switch, no normalize op): the context
psum is seeded with -sum_t H_t (fp16 residual pair) so unnormalized r
works directly, and 1/Z = 1/(sum r - T) is computed off-path. alpha*H
runs full-width split across DVE (4x tensor_scalar) / ACT (scale AP) /
Pool (one fused op), PE identity-accumulates into the seeded psum.
Gates + LSTM pointwise run per half in the transposed layout with one
fused sigmoid over i|f|o; gate psum is (half, mg, b')-major so each half
owns disjoint banks. Generator matmuls are emitted at the top of the
next step so the PE queue reaches them while stalled on alpha.

Host-side prep (numpy): fp16 casts, batch_H transpose for the projection
matmul, one-hot text encoding, gate columns permuted keras (i,f,c,o) ->
(i,f,o,c), b_lstm folded into the one-hot weight rows (valid because
one-hot rows sum to 1), bg added to the final output.
"""

import sys

for _p in ("/opt/trn_rl_repo", "/root/.axon_site/_ro/trn_rl_repo"):
    if _p not in sys.path:
        sys.path.insert(0, _p)

import numpy as np

import concourse.bass as bass
import concourse.tile as tile
from concourse import mybir
from concourse.bass_utils import run_bass_kernel_spmd
from concourse.masks import make_identity

dt = mybir.dt
AF = mybir.ActivationFunctionType
ALU = mybir.AluOpType

NCORES = 8
B, T, D, H, C = 1024, 64, 256, 256, 96
S = 26  # num steps = batch_max_length + 1
BC = B // NCORES  # 128 batch rows per core
KT = 2  # 256 = 2 x 128 tiles for d/h contraction
GT = 8  # 4H = 1024 = 8 m-tiles of 128
TB = BC * T  # 8192, flat (b, t) b-major
NCHUNK = 512  # psum-bank-limited matmul N
EWAVE = 2  # e-matmul psum slots are 2 banks (eB/eA2); the transposed
# gates psums multiplex onto eB (all uses sequential within a step),
# single-chunk waves ride the 1-bank "gen" slot, and the last 2-chunk
# wave borrows the misc slot (free during the z phase): 2+2+2+1 banks.
POOL_T = 8  # trailing t's of the alpha*H chain handled by one Pool op
# (Pool Multiply runs at 0.42 efficiency: 8 t's ~ 4.2us; Pool does no
# other per-step work since it cannot touch PSUM)

_CACHE = {}


def _split_excess_waits(nc, max_waits=1):
    """This container's walrus rejects instructions carrying more than
    ~max_waits semaphore waits ("Too many sync wait commands"). Hoist excess
    waits onto InstNoOp instructions inserted just before, on the same engine
    (per-engine program order makes this semantics-preserving)."""
    nid = [0]
    for f in nc.m.functions:
        for blk in f.blocks:
            insts = blk.instructions
            out = []
            changed = False
            for ins in insts:
                si = ins.sync_info
                ow = list(si.on_wait) if si is not None and si.on_wait else []
                if len(ow) > max_waits:
                    changed = True
                    while len(ow) > max_waits:
                        take, ow = ow[:max_waits], ow[max_waits:]
                        nid[0] += 1
                        nop = mybir.InstNoOp(
                            name=f"WSPLIT-{nid[0]}", engine=ins.engine,
                            sync_info=mybir.SyncInfo(on_wait=take,
                                                     on_update=[]))
                        nc.register_instruction(nop, overwrite=True)
                        out.append(nop)
                    ins.sync_info = mybir.SyncInfo(
                        on_wait=ow, on_update=list(si.on_update or []))
                out.append(ins)
            if changed:
                blk.instructions = out


def _build():
    nc = bass.Bass("TRN2", target_bir_lowering=False)
    f16, f32 = dt.float16, dt.float32

    h_nat_d = nc.declare_dram_parameter("h_nat", [BC, T, D], f16, isOutput=False)
    h_t_d = nc.declare_dram_parameter("h_t", [D, BC, T], f16, isOutput=False)
    oneh_d = nc.declare_dram_parameter("onehot", [C, S, BC], f16, isOutput=False)
    wi2h_d = nc.declare_dram_parameter("wi2h", [D, H], f16, isOutput=False)
    wh2h_d = nc.declare_dram_parameter("wh2h", [H, H], f16, isOutput=False)
    bh2h_d = nc.declare_dram_parameter("bh2hT", [H, 1], f32, isOutput=False)
    wsc_d = nc.declare_dram_parameter("wsc", [H, 128], f16, isOutput=False)
    wxc_d = nc.declare_dram_parameter("wxc", [D, 4 * H], f16, isOutput=False)
    wxo_d = nc.declare_dram_parameter("wxo", [C, 4 * H], f16, isOutput=False)
    wh_d = nc.declare_dram_parameter("wh", [H, 4 * H], f16, isOutput=False)
    wg_d = nc.declare_dram_parameter("wg", [H, C], f16, isOutput=False)
    probs_d = nc.declare_dram_parameter("probsT", [C, S, BC], f32, isOutput=True)

    with tile.TileContext(nc) as tc:
        import contextlib
        ctx = contextlib.ExitStack()
        with ctx:
            singles = ctx.enter_context(tc.tile_pool(name="singles", bufs=1))
            # psA: two 2-bank e-matmul wave slots (eB/eA2) so the PE can
            # fill one while the other drains; the transposed gates psums
            # multiplex onto eB (all uses sequential within a step).
            psA = ctx.enter_context(tc.tile_pool(name="psA", bufs=1, space="PSUM"))
            psB = ctx.enter_context(tc.tile_pool(name="psB", bufs=1, space="PSUM"))

            # ---- persistent SBUF state ----
            h_nat = singles.tile([BC, T, D], f16)
            projT = singles.tile([128, KT, BC, T], f16)  # [h', m, b, t]
            hidT = singles.tile([128, KT, S + 1, BC], f16)  # h states, slot 0 = 0
            oneh = singles.tile([C, S, BC], f16)
            wi2h = singles.tile([128, KT, H], f16)
            wh2h = singles.tile([128, KT, H], f16)
            bh2hT = singles.tile([128, KT, 1], f32)
            wsc = singles.tile([128, KT, 128], f16)
            wxc = singles.tile([128, KT, 4 * H], f16)
            wxo = singles.tile([C, 4 * H], f16)
            wh = singles.tile([128, KT, 4 * H], f16)
            wg = singles.tile([128, KT, C], f16)
            ident = singles.tile([128, 128], f16)
            cT = singles.tile([128, KT, BC], f16)
            probs_sb = singles.tile([C, S, BC], f32)

            # ---- load everything ----
            # (h_nat is issued after the h_tt chunks below: it is not read
            # until step 0's context phase, but 4 MB at the front of the
            # sync queue would delay the proj-critical h_tt transfer)
            nc.sync.dma_start(out=oneh, in_=oneh_d[:])
            nc.sync.dma_start(
                out=wi2h, in_=wi2h_d[:].rearrange("(k p) h -> p k h", p=128))
            nc.sync.dma_start(
                out=wh2h, in_=wh2h_d[:].rearrange("(k p) h -> p k h", p=128))
            nc.sync.dma_start(
                out=bh2hT, in_=bh2h_d[:].rearrange("(k p) o -> p k o", p=128))
            nc.sync.dma_start(
                out=wsc, in_=wsc_d[:].rearrange("(k p) o -> p k o", p=128))
            nc.sync.dma_start(
                out=wxc, in_=wxc_d[:].rearrange("(k p) g -> p k g", p=128))
            nc.sync.dma_start(out=wxo, in_=wxo_d[:])
            nc.sync.dma_start(
                out=wh, in_=wh_d[:].rearrange("(k p) g -> p k g", p=128))
            nc.sync.dma_start(
                out=wg, in_=wg_d[:].rearrange("(k p) c -> p k c", p=128))
            make_identity(nc, ident)
            nc.vector.memset(hidT[:, :, 0, :], 0.0)
            nc.vector.memset(cT, 0.0)

            # ---- precompute projT = (batch_H @ Wi2h)^T + bh2h : [h',m,(b t)]
            projT_f = projT[:].rearrange("p m b t -> p m (b t)")
            with tc.tile_pool(name="ht", bufs=1) as ht_pool:
                h_tt = ht_pool.tile([128, KT, BC, T], f16)
                # load in 4 b-chunks so the first proj matmuls start after a
                # quarter of the transfer instead of all of it
                for lb in range(4):
                    bls = slice(lb * (BC // 4), (lb + 1) * (BC // 4))
                    nc.sync.dma_start(
                        out=h_tt[:, :, bls, :],
                        in_=h_t_d[:, bls, :].rearrange(
                            "(k p) b t -> p k b t", p=128))
                nc.sync.dma_start(out=h_nat, in_=h_nat_d[:])
                h_tt_f = h_tt[:].rearrange("p k b t -> p k (b t)")
                nchk = TB // NCHUNK
                ncw = (nchk + EWAVE - 1) // EWAVE
                for m in range(KT):
                    for w in range(ncw):
                        nb = min(EWAVE, nchk - w * EWAVE)
                        ps = psA.tile([128, EWAVE, NCHUNK], f32,
                                      tag=("eB" if w % 2 == 0 else "eA2"))
                        for j in range(nb):
                            sl = slice((w * EWAVE + j) * NCHUNK,
                                       (w * EWAVE + j + 1) * NCHUNK)
                            for k in range(KT):
                                nc.tensor.matmul(
                                    ps[:, j, :],
                                    wi2h[:, k, m * 128:(m + 1) * 128],
                                    h_tt_f[:, k, sl], start=(k == 0),
                                    stop=(k == KT - 1))
                        sl3 = slice(w * EWAVE * NCHUNK,
                                    (w * EWAVE + nb) * NCHUNK)
                        pin = ps[:, :nb, :].rearrange("p a n -> p (a n)")
                        # fold the bh2h bias in during the drain, DVE/ACT
                        # alternating (Identity, unlike Copy, takes an AP
                        # bias; Pool can't read psum) - preamble only
                        if w % 2 == 0:
                            nc.vector.tensor_scalar(
                                out=projT_f[:, m, sl3], in0=pin,
                                scalar1=bh2hT[:, m, :], scalar2=None,
                                op0=ALU.add)
                        else:
                            nc.scalar.activation(
                                out=projT_f[:, m, sl3], in_=pin,
                                func=AF.Identity, bias=bh2hT[:, m, :])

            work = ctx.enter_context(tc.tile_pool(name="work", bufs=2))
            small = ctx.enter_context(tc.tile_pool(name="small", bufs=2))
            ahp = ctx.enter_context(tc.tile_pool(name="ahp", bufs=16))

            # ---- Hsum = sum_t H[:, t, :] (for the unnormalized-alpha
            # context trick: sum_t r_t H_t - Hsum = sum_t alpha_t H_t with
            # r = 1/sig(-e) = alpha + 1). Stored negated as an fp16
            # residual pair so the psum pre-load is fp32-exact.
            negA = singles.tile([BC, D], f16)
            negB = singles.tile([BC, D], f16)
            # dedicated drain buffer for the final e chunk: the rolling esb
            # pool would serialize the tail behind older waves' drains/DMAs
            esb15 = singles.tile([128, NCHUNK], f16)
            ps_hs = psB.tile([BC, D], dt.float32, tag="misc")
            for t in range(T):
                nc.tensor.matmul(ps_hs, ident, h_nat[:, t, :],
                                 start=(t == 0), stop=(t == T - 1))
            nc.scalar.activation(out=negA, in_=ps_hs, func=AF.Copy,
                                 scale=-1.0)
            hs32 = singles.tile([BC, D], dt.float32)
            nc.vector.tensor_scalar(out=hs32, in0=ps_hs, scalar1=-1.0,
                                    scalar2=None, op0=ALU.mult)
            nc.vector.tensor_tensor(out=negB, in0=hs32, in1=negA,
                                    op=ALU.subtract)

            # ---- decode steps ----
            def emit_gen(s):
                # generator matmuls for step s (reads hidT slot s+1); emitted
                # at the top of step s+1 so the PE queue reaches them while
                # stalled on step s+1's alpha (fills the tail gap). The
                # per-step output DMA hides the 1.3 MB store under compute.
                ps_p = psB.tile([C, 128], f32, tag="gen")
                for k in range(KT):
                    nc.tensor.matmul(
                        ps_p, wg[:, k, :], hidT[:, k, s + 1, :],
                        start=(k == 0), stop=(k == KT - 1))
                eng = nc.scalar if (s % 2 == 0) else nc.vector
                if eng is nc.scalar:
                    eng.copy(out=probs_sb[:, s, :], in_=ps_p)
                else:
                    eng.tensor_copy(probs_sb[:, s, :], ps_p)
                nc.sync.dma_start(out=probs_d[:, s, :], in_=probs_sb[:, s, :])

            # ---- half-pipelined decode ----
            # The recurrence is independent per batch row, and in both the
            # score layout (partitions = h') and the transposed gate/LSTM
            # layout (partitions = g') the batch lives on the free axis, so
            # splitting b into halves halves those ops' cost. Half A's
            # gates->LSTM->hp chain finishes first and its z(s+1) stream
            # starts while half B is still in its step-s tail; only the
            # full-width phases (alpha*H chain, ctx) need both halves.
            esbT = [singles.tile([128, 2, NCHUNK], f16, name=f"esbT{i}")
                    for i in range(4)]
            hpRs = {}

            def emit_hp_half(s, hi):
                # hp = h @ Wh2h for one b-half, transposed [h', m, b'];
                # drained straight into the 2x-replicated form the z-add's
                # broadcast AP wants (last dim step-1)
                hsl = slice(hi * 64, hi * 64 + 64)
                ps_hp = psB.tile([128, KT, 64], f32, tag="misc")
                for m in range(KT):
                    for k in range(KT):
                        nc.tensor.matmul(
                            ps_hp[:, m, :],
                            wh2h[:, k, m * 128:(m + 1) * 128],
                            hidT[:, k, s, hsl], start=(k == 0),
                            stop=(k == KT - 1))
                hpR = small.tile([128, KT, 64, 2], f16, tag=f"hpR{hi}")
                for m in range(KT):
                    base = ps_hp[:, m, :]
                    nc.vector.tensor_copy(
                        hpR[:, m, :, :],
                        bass.AP(tensor=base.tensor, offset=base.offset,
                                ap=[base.ap[0], [base.ap[-1][0], 64],
                                    [0, 2]]))
                hpRs[hi] = hpR

            for hi in (0, 1):
                emit_hp_half(0, hi)

            for s in range(S):
                if s > 0:
                    emit_gen(s - 1)

                alpha_e = small.tile([BC, T], f16, tag="alphae")
                sden = small.tile([BC, T], f16, tag="sden")
                rsd = small.tile([BC, T], f32, tag="rsd")

                def emit_softmax(h0, h1):
                    # r = 1/sigmoid(-e) = exp(e) + 1: unnormalized alpha
                    # offset by +1; the context psum pre-subtracts Hsum to
                    # compensate. sig(-e) is exact via the resident sigmoid
                    # table; no exp table switch.
                    nc.scalar.activation(out=sden[h0:h1, :],
                                         in_=alpha_e[h0:h1, :],
                                         func=AF.Sigmoid, scale=-1.0)
                    with nc.allow_low_precision(
                            reason="sig(-e) in [0.05, 0.95]; fp16 adds "
                                   "~5e-4 rel to alpha, under the 2e-2 "
                                   "budget"):
                        nc.vector.reciprocal(out=rsd[h0:h1, :],
                                             in_=sden[h0:h1, :])

                # z = projT + hp (bcast over t) per half; tanh on ACT;
                # e = w . tanh per 512-col chunk into psum waves, drained
                # fp16 and DMA-scattered to [b, t]. Single-chunk waves ride
                # dedicated buffers + the 1-bank gen slot so the per-half
                # tails never wait on rolling resources.
                ths = {}

                def z_groups(hi, groups, b0, upto=None):
                    base_b = hi * 64
                    hpR = hpRs[hi]
                    first = True
                    for nbz in groups:
                        bsl = slice(b0, b0 + nbz)
                        for m in range(KT):
                            # half B's first-group m0 add rides the idle
                            # Pool engine: its tanh slot comes after the
                            # whole tanh-A stream, so Pool's 3.3x-slower
                            # add is fully hidden. It gets its own buffer
                            # so its long hold doesn't stall the z ring.
                            pool_z = (hi == 1 and m == 0
                                      and nbz in (32, 24))
                            z = work.tile([128, 32, T], f16,
                                          tag="zp" if pool_z else "z")
                            hb = hpR[:, m, b0 - base_b:, :]
                            eng = nc.gpsimd if pool_z else nc.vector
                            eng.tensor_tensor(
                                out=z[:, :nbz, :].rearrange(
                                    "p b (r i) -> p b r i", i=2),
                                in0=projT[:, m, bsl, :].rearrange(
                                    "p b (r i) -> p b r i", i=2),
                                in1=bass.AP(
                                    tensor=hb.tensor, offset=hb.offset,
                                    ap=[hb.ap[0], [hb.ap[1][0], nbz],
                                        [0, T // 2], [1, 2]]),
                                op=ALU.add)
                            first = False
                            th = work.tile([128, 32 * T], f16,
                                           tag=f"th{hi}{m}")
                            nc.scalar.activation(
                                out=th[:, :nbz * T],
                                in_=z[:, :nbz, :].rearrange(
                                    "p b t -> p (b t)"),
                                func=AF.Tanh)
                            for q in range(nbz * T // NCHUNK):
                                c = (b0 - base_b) // 8 + q + hi * 8
                                ent = ths.get(c, [None, None, q * NCHUNK])
                                ent[m] = th
                                ent[2] = q * NCHUNK
                                ths[c] = ent
                        b0 += nbz
                    return b0

                def e_mms(chunks, tag):
                    nb = len(chunks)
                    if tag == "gen":
                        ps_e = psB.tile([128, 1, NCHUNK], f32, tag="gen")
                    elif tag == "misc":
                        ps_e = psB.tile([128, 2, NCHUNK], f32, tag="misc")
                    else:
                        ps_e = psA.tile([128, EWAVE, NCHUNK], f32, tag=tag)
                    for j, c in enumerate(chunks):
                        th0, th1, off = ths[c]
                        thm = (th0, th1)
                        for m in range(KT):
                            nc.tensor.matmul(
                                ps_e[:, j, :], wsc[:, m, :],
                                thm[m][:, off:off + NCHUNK],
                                start=(m == 0), stop=(m == KT - 1))
                    return ps_e

                def e_out(chunks, ps_e, ded, deng, dma_eng=None):
                    nb = len(chunks)
                    if ded is not None:
                        ebuf = esbT[ded]
                        tgt = ebuf[:, :nb, :]
                    else:
                        ebuf = work.tile([128, EWAVE, NCHUNK], f16,
                                         tag="esb")
                        tgt = ebuf[:, :nb, :]
                    if deng == "act":
                        nc.scalar.copy(out=tgt, in_=ps_e[:, :nb, :])
                    else:
                        nc.vector.tensor_copy(tgt, ps_e[:, :nb, :])
                    eb = ebuf[0:1, 0:nb, :]
                    (dma_eng or nc.sync).dma_start(
                        out=alpha_e[chunks[0] * 8:
                                    chunks[0] * 8 + nb * 8, :],
                        in_=bass.AP(tensor=eb.tensor, offset=eb.offset,
                                    ap=[eb.ap[0], [1, nb * NCHUNK]]),
                        single_packet=True)

                # --- half A: z stream + waves, inline drains (its tail is
                # hidden under half B's z stream) ---
                b0 = z_groups(0, (8, 24, 32), 0)
                assert b0 == 64
                for chunks, tag, ded, deng in (
                        ((0,), "gen", 0, "dve"),
                        ((1, 2), "eB", None, "dve"),
                        ((3, 4), "eA2", None, "dve"),
                        ((5, 6), "eB", None, "dve"),
                        ((7,), "gen", 1, "act")):
                    ps_e = e_mms(chunks, tag)
                    e_out(chunks, ps_e, ded, deng)
                emit_softmax(0, 64)

                # --- half B: the (96,128) tail gates the whole ah phase,
                # so chunks 12-15 drain first; the last wave rides the
                # misc psum slot (free during the z phase) and its drain +
                # DMA issue + sigmoid all stay on the ACT FIFO ---
                b0 = z_groups(1, (32, 24), 64)
                ps8 = e_mms((8,), "gen")
                e_out((8,), ps8, 2, "dve")
                ps910 = e_mms((9, 10), "eA2")
                ps11 = e_mms((11,), "gen")
                b0 = z_groups(1, (8,), b0)
                assert b0 == BC
                ps1213 = e_mms((12, 13), "eB")
                ps1415 = e_mms((14, 15), "misc")
                e_out((12, 13), ps1213, None, "dve")
                e_out((14, 15), ps1415, 3, "act", dma_eng=nc.scalar)
                emit_softmax(96, BC)
                e_out((9, 10), ps910, None, "dve")
                e_out((11,), ps11, 0, "dve")
                emit_softmax(64, 96)

                # gates partial (h and onehot contributions don't need ctx);
                # transposed: out [g', b], full width. No data deps, so the
                # PE chews it during the softmax tail, keeping the pstate
                # ramp alive for the ident accumulates. Each m-tile's psum
                # group is closed immediately (one open accumulation group
                # per psum bank) and drained to SBUF; re-injected per half.
                ps_gp = psA.tile([128, GT, 128], f32, tag="eB")
                for mg in range(GT):
                    gsl = slice(mg * 128, (mg + 1) * 128)
                    for k in range(KT):
                        nc.tensor.matmul(
                            ps_gp[:, mg, :], wh[:, k, gsl],
                            hidT[:, k, s, :], start=(k == 0), stop=False)
                    nc.tensor.matmul(
                        ps_gp[:, mg, :], wxo[:, gsl], oneh[:, s, :],
                        start=False, stop=True)
                ghosb = work.tile([128, GT, 128], f16, tag="gho")
                nc.vector.tensor_copy(ghosb, ps_gp)

                # 1/Z for the later ctx rescale: Z = sum_t r - T, off the
                # critical path (only needed once the ah chain finishes)
                zs = small.tile([BC, 1], f32, tag="zsum")
                nc.vector.reduce_sum(zs, rsd, mybir.AxisListType.X)
                zm = small.tile([BC, 1], f32, tag="zm")
                nc.vector.tensor_scalar(out=zm, in0=zs, scalar1=-float(T),
                                        scalar2=None, op0=ALU.add)
                rzb = small.tile([BC, 1], f32, tag="rzb")
                nc.vector.reciprocal(out=rzb, in_=zm)

                # context = sum_t r[:, t] * H[:, t, :] - Hsum (r = alpha+1),
                # full width: r_t*H_t slices at 4x on DVE with ACT every 5th
                # t and Pool one fused op for the last POOL_T t's; PE
                # accumulates via identity matmuls into psum seeded with
                # -Hsum (fp16 residual pair, no data deps).
                ps_cx = psB.tile([BC, D], f32, tag="misc")
                nc.tensor.matmul(ps_cx, ident, negA, start=True, stop=False)
                nc.tensor.matmul(ps_cx, ident, negB, start=False, stop=False)
                TS = T - POOL_T
                ah_pool = work.tile([BC, POOL_T, D], f16, tag="ahpool")
                hb = h_nat[:, TS:, :]
                al = rsd[:, TS:]
                nc.gpsimd.tensor_tensor(
                    out=ah_pool, in0=hb,
                    in1=bass.AP(tensor=al.tensor, offset=al.offset,
                                ap=[al.ap[0], [al.ap[-1][0], POOL_T],
                                    [0, D]]),
                    op=ALU.mult)
                ahs = []
                for t in range(TS):
                    ah = ahp.tile([BC, D], f16, tag="ah")
                    if t % 5 == 4:
                        nc.scalar.activation(
                            out=ah, in_=h_nat[:, t, :], func=AF.Copy,
                            scale=rsd[:, t:t + 1])
                    else:
                        nc.vector.tensor_scalar(
                            out=ah, in0=h_nat[:, t, :],
                            scalar1=rsd[:, t:t + 1],
                            scalar2=None, op0=ALU.mult)
                    ahs.append(ah)
                    if t % 16 == 15 or t == TS - 1:
                        for a in ahs:
                            nc.tensor.matmul(ps_cx, ident, a, start=False,
                                             stop=False)
                        ahs = []
                assert not ahs
                for j in range(POOL_T):
                    nc.tensor.matmul(ps_cx, ident, ah_pool[:, j, :],
                                     start=False, stop=(j == POOL_T - 1))
                # 1/Z rescale on ACT (Copy takes a per-partition AP scale
                # and reads psum through the faster port)
                ctxv = small.tile([BC, D], f16, tag="ctxv")
                nc.scalar.activation(out=ctxv, in_=ps_cx, func=AF.Copy,
                                     scale=rzb[:, 0:1])
                # transpose context -> [d', m, b]
                ps_ct = psB.tile([BC, KT, 128], f16, tag="misc")
                ctxT = small.tile([128, KT, 128], f16, tag="ctxT")
                for m in range(KT):
                    nc.tensor.transpose(
                        ps_ct[:, m, :], ctxv[:, m * 128:(m + 1) * 128],
                        ident)
                    nc.vector.tensor_copy(ctxT[:, m, :], ps_ct[:, m, :])

                # per-half tail: gates -> LSTM -> hp. Emission order keeps
                # each engine FIFO clean: gatesA, gatesB (PE) / lstmA,
                # lstmB (ACT+DVE) / hpA, hpB (PE) with the hpR drains last,
                # so next step's half-A z stream starts while half B
                # finishes. Gate psum is (half, mg, b')-major: each half
                # owns disjoint banks, so B's accumulation never collides
                # with A's LSTM reads.
                ps_g = psA.tile([128, 2, GT, 64], f32, tag="eB")
                for hi in (0, 1):
                    hsl = slice(hi * 64, hi * 64 + 64)
                    for mg in range(GT):
                        gsl = slice(mg * 128, (mg + 1) * 128)
                        nc.tensor.matmul(
                            ps_g[:, hi, mg, :], ident, ghosb[:, mg, hsl],
                            start=True, stop=False)
                        for k in range(KT):
                            nc.tensor.matmul(
                                ps_g[:, hi, mg, :], wxc[:, k, gsl],
                                ctxT[:, k, hsl],
                                start=False, stop=(k == KT - 1))
                for hi in (0, 1):
                    hsl = slice(hi * 64, hi * 64 + 64)
                    # LSTM pointwise, transposed [g', b']. permuted gate
                    # order i,f,o,g -> m-tiles 0:2 = i, 2:4 = f, 4:6 = o,
                    # 6:8 = g (b_lstm folded into wxo host-side).
                    sig_ifo = small.tile([128, 6, 64], f16, tag="sig_ifo")
                    tg = small.tile([128, KT, 64], f16, tag="tg")
                    nc.scalar.activation(out=sig_ifo,
                                         in_=ps_g[:, hi, 0:6, :],
                                         func=AF.Sigmoid)
                    nc.scalar.activation(out=tg, in_=ps_g[:, hi, 6:8, :],
                                         func=AF.Tanh)
                    t1 = small.tile([128, KT, 64], f16, tag="t1")
                    t2 = small.tile([128, KT, 64], f16, tag="t2")
                    nc.vector.tensor_tensor(out=t1, in0=sig_ifo[:, 2:4, :],
                                            in1=cT[:, :, hsl], op=ALU.mult)
                    nc.vector.tensor_tensor(out=t2, in0=sig_ifo[:, 0:2, :],
                                            in1=tg, op=ALU.mult)
                    nc.vector.tensor_tensor(out=cT[:, :, hsl], in0=t1,
                                            in1=t2, op=ALU.add)
                    tc_t = small.tile([128, KT, 64], f16, tag="tc_t")
                    nc.scalar.activation(out=tc_t, in_=cT[:, :, hsl],
                                         func=AF.Tanh)
                    # h goes straight into hidT layout - no transpose
                    nc.vector.tensor_tensor(out=hidT[:, :, s + 1, hsl],
                                            in0=sig_ifo[:, 4:6, :],
                                            in1=tc_t, op=ALU.mult)
                    if s < S - 1:
                        emit_hp_half(s + 1, hi)

            emit_gen(S - 1)

    _split_excess_waits(nc)
    return nc


def _get_module():
    if "nc" not in _CACHE:
        _CACHE["nc"] = _build()
    return _CACHE["nc"]


def build_in_maps(batch_H, text, batch_max_length, Wi2h, Wh2h, bh2h, w_score,
                  Wx, Wh, b_lstm, Wg, bg):
    batch_H = np.asarray(batch_H, dtype=np.float32)
    text = np.asarray(text)
    assert int(batch_max_length) + 1 == S
    assert batch_H.shape == (B, T, D)

    f16 = np.float16
    bh16 = batch_H.astype(f16)
    # one-hot text: [B, S, C] -> per-core [S, C, BC]
    oh = (text[:, :S, None] == np.arange(C)[None, None, :])

    # permute keras gate order (i, f, c, o) -> (i, f, o, c)
    perm = np.concatenate([np.arange(0, 2 * H),          # i, f
                           np.arange(3 * H, 4 * H),      # o
                           np.arange(2 * H, 3 * H)])     # c/g
    Wx = np.asarray(Wx, np.float32)[:, perm]
    Wh_p = np.asarray(Wh, np.float32)[:, perm]
    bl_p = np.asarray(b_lstm, np.float32)[perm]
    wxo_p = (Wx[D:D + C, :] + bl_p[None, :]).astype(f16)
    weights = {
        "wi2h": np.ascontiguousarray(np.asarray(Wi2h, np.float32).astype(f16)),
        "wh2h": np.ascontiguousarray(np.asarray(Wh2h, np.float32).astype(f16)),
        "bh2hT": np.ascontiguousarray(
            np.asarray(bh2h, np.float32).reshape(H, 1)),
        "wsc": np.ascontiguousarray(np.tile(
            np.asarray(w_score, np.float32).reshape(H, 1), (1, 128)).astype(f16)),
        "wxc": np.ascontiguousarray(Wx[:D, :].astype(f16)),
        "wxo": np.ascontiguousarray(wxo_p),
        "wh": np.ascontiguousarray(Wh_p.astype(f16)),
        "wg": np.ascontiguousarray(np.asarray(Wg, np.float32).astype(f16)),
    }

    in_maps = []
    for c in range(NCORES):
        bsl = slice(c * BC, (c + 1) * BC)
        in_maps.append({
            "h_nat": np.ascontiguousarray(bh16[bsl]),
            "h_t": np.ascontiguousarray(bh16[bsl].transpose(2, 0, 1)),
            "onehot": np.ascontiguousarray(
                oh[bsl].transpose(2, 1, 0).astype(f16)),
            **weights,
        })
    return in_maps


def kernel(**inputs):
    in_maps = build_in_maps(**inputs)
    bg = inputs["bg"]

    nc = _get_module()
    res = run_bass_kernel_spmd(nc, in_maps, list(range(NCORES)))

    out = np.empty((B, S, C), np.float32)
    for c in range(NCORES):
        out[c * BC:(c + 1) * BC] = res.results[c]["probsT"].transpose(2, 1, 0)
    out += np.asarray(bg, np.float32)[None, None, :]
    return out


if __name__ == "__main__":
    _build()
    print("build OK")


# revision 52
# speedup vs baseline: 1.1233x; 1.0809x over previous
"""Trainium2 Bass kernel for an attention-LSTM decoder (Bahdanau attention +
LSTM cell + generator head), data-parallel over 8 NeuronCores.

Shapes (hardcoded): B=1024, T=64, D=256, H=256, C=96, steps S=26.
Each core handles 128 batch rows.

Device layouts (per core, b = 128):
  - score chain runs "transposed": partitions = hidden dim tiles (2 x 128),
    free = (b, t) flat b-major.
  - softmax/context run natural: partitions = b, free = t / d.
  - gates + LSTM pointwise run TRANSPOSED (partitions = gate-dim tiles,
    free = b) so h lands directly in hidT layout - no h transpose.
Matmul operands are fp16 (full-rate PE streaming); PSUM accumulation is
fp32; the recurrent c state is fp16 (transposed).

The recurrence is independent per batch row, so each step is split into
two b-halves that pipeline through the engines: half A's gates -> LSTM ->
hp chain finishes first and its z(s+1) stream starts while half B is
still in its step-s tail. Only the alpha*H chain and context need full
width. Per-step pipeline (per half): z = projT + hp (DVE, bcast AP; bh2h
pre-folded into projT; hp drained from psum directly in 2x-replicated
form) -> tanh (ACT) -> e-matmul (M=128-replicated w_score) into 2-bank
psum waves -> drain fp16 -> DMA-scatter to [b, t]. The (96,128) tail
gates the whole next phase, so chunks 12-15 drain first, the last wave
borrows the misc psum slot (free during the z phase) and keeps drain +
DMA issue + sigmoid on one ACT FIFO. Softmax r = 1/sigmoid(-e) = alpha+1
(resident sigmoid table, no exp switch, no normalize op): the context
psum is seeded with -sum_t H_t (fp16 residual pair) so unnormalized r
works directly, and 1/Z = 1/(sum r - T) is computed off-path. alpha*H
runs full-width split across DVE (4x tensor_scalar) / ACT (scale AP) /
Pool (one fused op), PE identity-accumulates into the seeded psum.
Gates + LSTM pointwise run per half in the transposed layout with one
fused sigmoid over i|f|o; gate psum is (half, mg, b')-major so each half
owns disjoint banks. Generator matmuls are emitted at the top of the
next step so the PE queue reaches them while stalled on alpha.

Host-side prep (numpy): fp16 casts, batch_H transpose for the projection
matmul, one-hot text encoding, gate columns permuted keras (i,f,c,o) ->
(i,f,o,c), b_lstm folded into the one-hot weight rows (valid because
one-hot rows sum to 1), bg added to the final output.
"""

import sys

for _p in ("/opt/trn_rl_repo", "/root/.axon_site/_ro/trn_rl_repo"):
    if _p not in sys.path:
        sys.path.insert(0, _p)

import numpy as np

import concourse.bass as bass
import concourse.tile as tile
from concourse import mybir
from concourse.bass_utils import run_bass_kernel_spmd
from concourse.masks import make_identity

dt = mybir.dt
AF = mybir.ActivationFunctionType
ALU = mybir.AluOpType

NCORES = 8
B, T, D, H, C = 1024, 64, 256, 256, 96
S = 26  # num steps = batch_max_length + 1
BC = B // NCORES  # 128 batch rows per core
KT = 2  # 256 = 2 x 128 tiles for d/h contraction
GT = 8  # 4H = 1024 = 8 m-tiles of 128
TB = BC * T  # 8192, flat (b, t) b-major
NCHUNK = 512  # psum-bank-limited matmul N
EWAVE = 2  # e-matmul psum slots are 2 banks (eB/eA2); the transposed
# gates psums multiplex onto eB (all uses sequential within a step),
# single-chunk waves ride the 1-bank "gen" slot, and the last 2-chunk
# wave borrows the misc slot (free during the z phase): 2+2+2+1 banks.
POOL_T = 8  # trailing t's of the alpha*H chain handled by one Pool op
# (Pool Multiply runs at 0.42 efficiency: 8 t's ~ 4.2us; Pool does no
# other per-step work since it cannot touch PSUM)

_CACHE = {}


def _split_excess_waits(nc, max_waits=1):
    """This container's walrus rejects instructions carrying more than
    ~max_waits semaphore waits ("Too many sync wait commands"). Hoist excess
    waits onto InstNoOp instructions inserted just before, on the same engine
    (per-engine program order makes this semantics-preserving)."""
    nid = [0]
    for f in nc.m.functions:
        for blk in f.blocks:
            insts = blk.instructions
            out = []
            changed = False
            for ins in insts:
                si = ins.sync_info
                ow = list(si.on_wait) if si is not None and si.on_wait else []
                if len(ow) > max_waits:
                    changed = True
                    while len(ow) > max_waits:
                        take, ow = ow[:max_waits], ow[max_waits:]
                        nid[0] += 1
                        nop = mybir.InstNoOp(
                            name=f"WSPLIT-{nid[0]}", engine=ins.engine,
                            sync_info=mybir.SyncInfo(on_wait=take,
                                                     on_update=[]))
                        nc.register_instruction(nop, overwrite=True)
                        out.append(nop)
                    ins.sync_info = mybir.SyncInfo(
                        on_wait=ow, on_update=list(si.on_update or []))
                out.append(ins)
            if changed:
                blk.instructions = out


def _build():
    nc = bass.Bass("TRN2", target_bir_lowering=False)
    f16, f32 = dt.float16, dt.float32

    h_nat_d = nc.declare_dram_parameter("h_nat", [BC, T, D], f16, isOutput=False)
    h_t_d = nc.declare_dram_parameter("h_t", [D, BC, T], f16, isOutput=False)
    oneh_d = nc.declare_dram_parameter("onehot", [C, S, BC], f16, isOutput=False)
    wi2h_d = nc.declare_dram_parameter("wi2h", [D, H], f16, isOutput=False)
    wh2h_d = nc.declare_dram_parameter("wh2h", [H, H], f16, isOutput=False)
    bh2h_d = nc.declare_dram_parameter("bh2hT", [H, 1], f32, isOutput=False)
    wsc_d = nc.declare_dram_parameter("wsc", [H, 128], f16, isOutput=False)
    wxc_d = nc.declare_dram_parameter("wxc", [D, 4 * H], f16, isOutput=False)
    wxo_d = nc.declare_dram_parameter("wxo", [C, 4 * H], f16, isOutput=False)
    wh_d = nc.declare_dram_parameter("wh", [H, 4 * H], f16, isOutput=False)
    wg_d = nc.declare_dram_parameter("wg", [H, C], f16, isOutput=False)
    probs_d = nc.declare_dram_parameter("probsT", [C, S, BC], f32, isOutput=True)

    with tile.TileContext(nc) as tc:
        import contextlib
        ctx = contextlib.ExitStack()
        with ctx:
            singles = ctx.enter_context(tc.tile_pool(name="singles", bufs=1))
            # psA: two 2-bank e-matmul wave slots (eB/eA2) so the PE can
            # fill one while the other drains; the transposed gates psums
            # multiplex onto eB (all uses sequential within a step).
            psA = ctx.enter_context(tc.tile_pool(name="psA", bufs=1, space="PSUM"))
            psB = ctx.enter_context(tc.tile_pool(name="psB", bufs=1, space="PSUM"))

            # ---- persistent SBUF state ----
            h_nat = singles.tile([BC, T, D], f16)
            projT = singles.tile([128, KT, BC, T], f16)  # [h', m, b, t]
            hidT = singles.tile([128, KT, S + 1, BC], f16)  # h states, slot 0 = 0
            oneh = singles.tile([C, S, BC], f16)
            wi2h = singles.tile([128, KT, H], f16)
            wh2h = singles.tile([128, KT, H], f16)
            bh2hT = singles.tile([128, KT, 1], f32)
            wsc = singles.tile([128, KT, 128], f16)
            wxc = singles.tile([128, KT, 4 * H], f16)
            wxo = singles.tile([C, 4 * H], f16)
            wh = singles.tile([128, KT, 4 * H], f16)
            wg = singles.tile([128, KT, C], f16)
            ident = singles.tile([128, 128], f16)
            cT = singles.tile([128, KT, BC], f16)
            # 4-slot ring: each step's probs are DMA'd out immediately
            probs_sb = singles.tile([C, 4, BC], f32)

            # ---- load everything ----
            # (h_nat is issued after the h_tt chunks below: it is not read
            # until step 0's context phase, but 4 MB at the front of the
            # sync queue would delay the proj-critical h_tt transfer)
            nc.sync.dma_start(out=oneh, in_=oneh_d[:])
            nc.sync.dma_start(
                out=wi2h, in_=wi2h_d[:].rearrange("(k p) h -> p k h", p=128))
            nc.sync.dma_start(
                out=wh2h, in_=wh2h_d[:].rearrange("(k p) h -> p k h", p=128))
            nc.sync.dma_start(
                out=bh2hT, in_=bh2h_d[:].rearrange("(k p) o -> p k o", p=128))
            nc.sync.dma_start(
                out=wsc, in_=wsc_d[:].rearrange("(k p) o -> p k o", p=128))
            nc.sync.dma_start(
                out=wxc, in_=wxc_d[:].rearrange("(k p) g -> p k g", p=128))
            nc.sync.dma_start(out=wxo, in_=wxo_d[:])
            nc.sync.dma_start(
                out=wh, in_=wh_d[:].rearrange("(k p) g -> p k g", p=128))
            nc.sync.dma_start(
                out=wg, in_=wg_d[:].rearrange("(k p) c -> p k c", p=128))
            make_identity(nc, ident)
            nc.vector.memset(hidT[:, :, 0, :], 0.0)
            nc.vector.memset(cT, 0.0)

            # ---- precompute projT = (batch_H @ Wi2h)^T + bh2h : [h',m,(b t)]
            projT_f = projT[:].rearrange("p m b t -> p m (b t)")
            with tc.tile_pool(name="ht", bufs=1) as ht_pool:
                h_tt = ht_pool.tile([128, KT, BC, T], f16)
                # load in 4 b-chunks so the first proj matmuls start after a
                # quarter of the transfer instead of all of it
                for lb in range(4):
                    bls = slice(lb * (BC // 4), (lb + 1) * (BC // 4))
                    nc.sync.dma_start(
                        out=h_tt[:, :, bls, :],
                        in_=h_t_d[:, bls, :].rearrange(
                            "(k p) b t -> p k b t", p=128))
                nc.sync.dma_start(out=h_nat, in_=h_nat_d[:])
                h_tt_f = h_tt[:].rearrange("p k b t -> p k (b t)")
                nchk = TB // NCHUNK
                ncw = (nchk + EWAVE - 1) // EWAVE
                for m in range(KT):
                    for w in range(ncw):
                        nb = min(EWAVE, nchk - w * EWAVE)
                        ps = psA.tile([128, EWAVE, NCHUNK], f32,
                                      tag=("eB" if w % 2 == 0 else "eA2"))
                        for j in range(nb):
                            sl = slice((w * EWAVE + j) * NCHUNK,
                                       (w * EWAVE + j + 1) * NCHUNK)
                            for k in range(KT):
                                nc.tensor.matmul(
                                    ps[:, j, :],
                                    wi2h[:, k, m * 128:(m + 1) * 128],
                                    h_tt_f[:, k, sl], start=(k == 0),
                                    stop=(k == KT - 1))
                        sl3 = slice(w * EWAVE * NCHUNK,
                                    (w * EWAVE + nb) * NCHUNK)
                        pin = ps[:, :nb, :].rearrange("p a n -> p (a n)")
                        # fold the bh2h bias in during the drain, DVE/ACT
                        # alternating (Identity, unlike Copy, takes an AP
                        # bias; Pool can't read psum) - preamble only
                        if w % 2 == 0:
                            nc.vector.tensor_scalar(
                                out=projT_f[:, m, sl3], in0=pin,
                                scalar1=bh2hT[:, m, :], scalar2=None,
                                op0=ALU.add)
                        else:
                            nc.scalar.activation(
                                out=projT_f[:, m, sl3], in_=pin,
                                func=AF.Identity, bias=bh2hT[:, m, :])

            work = ctx.enter_context(tc.tile_pool(name="work", bufs=2))
            small = ctx.enter_context(tc.tile_pool(name="small", bufs=2))
            ahp = ctx.enter_context(tc.tile_pool(name="ahp", bufs=16))

            # ---- Hsum = sum_t H[:, t, :] (for the unnormalized-alpha
            # context trick: sum_t r_t H_t - Hsum = sum_t alpha_t H_t with
            # r = 1/sig(-e) = alpha + 1). Stored negated as an fp16
            # residual pair so the psum pre-load is fp32-exact.
            negA = singles.tile([BC, D], f16)
            negB = singles.tile([BC, D], f16)
            # dedicated drain buffer for the final e chunk: the rolling esb
            # pool would serialize the tail behind older waves' drains/DMAs
            esb15 = singles.tile([128, NCHUNK], f16)
            ps_hs = psB.tile([BC, D], dt.float32, tag="misc")
            for t in range(T):
                nc.tensor.matmul(ps_hs, ident, h_nat[:, t, :],
                                 start=(t == 0), stop=(t == T - 1))
            nc.scalar.activation(out=negA, in_=ps_hs, func=AF.Copy,
                                 scale=-1.0)
            hs32 = singles.tile([BC, D], dt.float32)
            nc.vector.tensor_scalar(out=hs32, in0=ps_hs, scalar1=-1.0,
                                    scalar2=None, op0=ALU.mult)
            nc.vector.tensor_tensor(out=negB, in0=hs32, in1=negA,
                                    op=ALU.subtract)

            # ---- decode steps ----
            def emit_gen(s):
                # generator matmuls for step s (reads hidT slot s+1); emitted
                # at the top of step s+1 so the PE queue reaches them while
                # stalled on step s+1's alpha (fills the tail gap). The
                # per-step output DMA hides the 1.3 MB store under compute.
                ps_p = psB.tile([C, 128], f32, tag="gen")
                for k in range(KT):
                    nc.tensor.matmul(
                        ps_p, wg[:, k, :], hidT[:, k, s + 1, :],
                        start=(k == 0), stop=(k == KT - 1))
                sl = s % 4
                if s % 2 == 0:
                    nc.scalar.copy(out=probs_sb[:, sl, :], in_=ps_p)
                else:
                    nc.vector.tensor_copy(probs_sb[:, sl, :], ps_p)
                nc.sync.dma_start(out=probs_d[:, s, :],
                                  in_=probs_sb[:, sl, :])

            # ---- half-pipelined decode ----
            # The recurrence is independent per batch row, and in both the
            # score layout (partitions = h') and the transposed gate/LSTM
            # layout (partitions = g') the batch lives on the free axis, so
            # splitting b into halves halves those ops' cost. Half A's
            # gates->LSTM->hp chain finishes first and its z(s+1) stream
            # starts while half B is still in its step-s tail; only the
            # full-width phases (alpha*H chain, ctx) need both halves.
            esbT = [singles.tile([128, 2, NCHUNK], f16, name=f"esbT{i}")
                    for i in range(4)]
            hpRs = {}

            def emit_hp_half(s, hi):
                # hp = h @ Wh2h for one b-half, transposed [h', m, b'];
                # drained straight into the 2x-replicated form the z-add's
                # broadcast AP wants (last dim step-1)
                hsl = slice(hi * 64, hi * 64 + 64)
                ps_hp = psB.tile([128, KT, 64], f32, tag="misc")
                for m in range(KT):
                    for k in range(KT):
                        nc.tensor.matmul(
                            ps_hp[:, m, :],
                            wh2h[:, k, m * 128:(m + 1) * 128],
                            hidT[:, k, s, hsl], start=(k == 0),
                            stop=(k == KT - 1))
                hpR = small.tile([128, KT, 64, 2], f16, tag=f"hpR{hi}")
                for m in range(KT):
                    base = ps_hp[:, m, :]
                    nc.vector.tensor_copy(
                        hpR[:, m, :, :],
                        bass.AP(tensor=base.tensor, offset=base.offset,
                                ap=[base.ap[0], [base.ap[-1][0], 64],
                                    [0, 2]]))
                hpRs[hi] = hpR

            for hi in (0, 1):
                emit_hp_half(0, hi)

            for s in range(S):
                if s > 0:
                    emit_gen(s - 1)

                alpha_e = small.tile([BC, T], f16, tag="alphae")
                sden = small.tile([BC, T], f16, tag="sden")
                rsd = small.tile([BC, T], f32, tag="rsd")

                def emit_softmax(h0, h1):
                    # r = 1/sigmoid(-e) = exp(e) + 1: unnormalized alpha
                    # offset by +1; the context psum pre-subtracts Hsum to
                    # compensate. sig(-e) is exact via the resident sigmoid
                    # table; no exp table switch.
                    nc.scalar.activation(out=sden[h0:h1, :],
                                         in_=alpha_e[h0:h1, :],
                                         func=AF.Sigmoid, scale=-1.0)
                    with nc.allow_low_precision(
                            reason="sig(-e) in [0.05, 0.95]; fp16 adds "
                                   "~5e-4 rel to alpha, under the 2e-2 "
                                   "budget"):
                        nc.vector.reciprocal(out=rsd[h0:h1, :],
                                             in_=sden[h0:h1, :])

                # z = projT + hp (bcast over t) per half; tanh on ACT;
                # e = w . tanh per 512-col chunk into psum waves, drained
                # fp16 and DMA-scattered to [b, t]. Single-chunk waves ride
                # dedicated buffers + the 1-bank gen slot so the per-half
                # tails never wait on rolling resources.
                ths = {}

                def z_groups(hi, groups, b0, upto=None):
                    base_b = hi * 64
                    hpR = hpRs[hi]
                    for nbz in groups:
                        bsl = slice(b0, b0 + nbz)
                        # both m-tiles of a group share one z buffer so a
                        # single tanh covers them (halves the ACT per-op
                        # SBUF-access overhead in the z-bound phase)
                        pool_g = hi == 1 and nbz in (32, 24)
                        z = work.tile([128, KT, 32, T], f16,
                                      tag="zp" if pool_g else "z")
                        for m in range(KT):
                            # half B's big-group m0 adds ride the idle
                            # Pool engine: their tanh slots come after the
                            # whole tanh-A stream, so Pool's 3.3x-slower
                            # add is fully hidden. Those groups get their
                            # own buffer tag so the long hold doesn't
                            # stall the z ring.
                            hb = hpR[:, m, b0 - base_b:, :]
                            eng = (nc.gpsimd if (pool_g and m == 0)
                                   else nc.vector)
                            eng.tensor_tensor(
                                out=z[:, m, :nbz, :].rearrange(
                                    "p b (r i) -> p b r i", i=2),
                                in0=projT[:, m, bsl, :].rearrange(
                                    "p b (r i) -> p b r i", i=2),
                                in1=bass.AP(
                                    tensor=hb.tensor, offset=hb.offset,
                                    ap=[hb.ap[0], [hb.ap[1][0], nbz],
                                        [0, T // 2], [1, 2]]),
                                op=ALU.add)
                        th = work.tile([128, KT, 32 * T], f16,
                                       tag=f"th{hi}")
                        nc.scalar.activation(
                            out=th[:, :, :nbz * T],
                            in_=z[:, :, :nbz, :].rearrange(
                                "p m b t -> p m (b t)"),
                            func=AF.Tanh)
                        for q in range(nbz * T // NCHUNK):
                            c = (b0 - base_b) // 8 + q + hi * 8
                            ths[c] = (th, q * NCHUNK)
                        b0 += nbz
                    return b0

                def e_mms(chunks, tag):
                    nb = len(chunks)
                    if tag == "gen":
                        ps_e = psB.tile([128, 1, NCHUNK], f32, tag="gen")
                    elif tag == "misc":
                        ps_e = psB.tile([128, 2, NCHUNK], f32, tag="misc")
                    else:
                        ps_e = psA.tile([128, EWAVE, NCHUNK], f32, tag=tag)
                    for j, c in enumerate(chunks):
                        th, off = ths[c]
                        for m in range(KT):
                            nc.tensor.matmul(
                                ps_e[:, j, :], wsc[:, m, :],
                                th[:, m, off:off + NCHUNK],
                                start=(m == 0), stop=(m == KT - 1))
                    return ps_e

                def e_out(chunks, ps_e, ded, deng, dma_eng=None):
                    nb = len(chunks)
                    if ded is not None:
                        ebuf = esbT[ded]
                        tgt = ebuf[:, :nb, :]
                    else:
                        ebuf = work.tile([128, EWAVE, NCHUNK], f16,
                                         tag="esb")
                        tgt = ebuf[:, :nb, :]
                    if deng == "act":
                        nc.scalar.copy(out=tgt, in_=ps_e[:, :nb, :])
                    else:
                        nc.vector.tensor_copy(tgt, ps_e[:, :nb, :])
                    eb = ebuf[0:1, 0:nb, :]
                    (dma_eng or nc.sync).dma_start(
                        out=alpha_e[chunks[0] * 8:
                                    chunks[0] * 8 + nb * 8, :],
                        in_=bass.AP(tensor=eb.tensor, offset=eb.offset,
                                    ap=[eb.ap[0], [1, nb * NCHUNK]]),
                        single_packet=True)

                # --- half A: z stream + waves, inline drains (its tail is
                # hidden under half B's z stream) ---
                b0 = z_groups(0, (8, 24, 32), 0)
                assert b0 == 64
                for chunks, tag, ded, deng in (
                        ((0,), "gen", 0, "dve"),
                        ((1, 2), "eB", None, "dve"),
                        ((3, 4), "eA2", None, "dve"),
                        ((5, 6), "eB", None, "dve"),
                        ((7,), "gen", 1, "act")):
                    ps_e = e_mms(chunks, tag)
                    e_out(chunks, ps_e, ded, deng)
                emit_softmax(0, 64)

                # --- half B: the (96,128) tail gates the whole ah phase,
                # so chunks 12-15 drain first; the last wave rides the
                # misc psum slot (free during the z phase) and its drain +
                # DMA issue + sigmoid all stay on the ACT FIFO ---
                b0 = z_groups(1, (32, 24), 64)
                ps8 = e_mms((8,), "gen")
                e_out((8,), ps8, 2, "dve")
                ps910 = e_mms((9, 10), "eA2")
                ps11 = e_mms((11,), "gen")
                b0 = z_groups(1, (8,), b0)
                assert b0 == BC
                ps1213 = e_mms((12, 13), "eB")
                ps1415 = e_mms((14, 15), "misc")
                e_out((12, 13), ps1213, None, "dve")
                e_out((14, 15), ps1415, 3, "act", dma_eng=nc.scalar)
                emit_softmax(96, BC)
                e_out((9, 10), ps910, None, "dve")
                e_out((11,), ps11, 0, "dve")
                emit_softmax(64, 96)

                # gates partial (h and onehot contributions don't need ctx);
                # transposed: out [g', b], full width. No data deps, so the
                # PE chews it during the softmax tail, keeping the pstate
                # ramp alive for the ident accumulates. Each m-tile's psum
                # group is closed immediately (one open accumulation group
                # per psum bank) and drained to SBUF; re-injected per half.
                ps_gp = psA.tile([128, GT, 128], f32, tag="eB")
                for mg in range(GT):
                    gsl = slice(mg * 128, (mg + 1) * 128)
                    for k in range(KT):
                        nc.tensor.matmul(
                            ps_gp[:, mg, :], wh[:, k, gsl],
                            hidT[:, k, s, :], start=(k == 0), stop=False)
                    nc.tensor.matmul(
                        ps_gp[:, mg, :], wxo[:, gsl], oneh[:, s, :],
                        start=False, stop=True)
                ghosb = work.tile([128, GT, 128], f16, tag="gho")
                nc.vector.tensor_copy(ghosb, ps_gp)

                # 1/Z for the later ctx rescale: Z = sum_t r - T, off the
                # critical path (only needed once the ah chain finishes)
                zs = small.tile([BC, 1], f32, tag="zsum")
                nc.vector.reduce_sum(zs, rsd, mybir.AxisListType.X)
                zm = small.tile([BC, 1], f32, tag="zm")
                nc.vector.tensor_scalar(out=zm, in0=zs, scalar1=-float(T),
                                        scalar2=None, op0=ALU.add)
                rzb = small.tile([BC, 1], f32, tag="rzb")
                nc.vector.reciprocal(out=rzb, in_=zm)

                # context = sum_t r[:, t] * H[:, t, :] - Hsum (r = alpha+1),
                # full width: r_t*H_t slices at 4x on DVE with ACT every 5th
                # t and Pool one fused op for the last POOL_T t's; PE
                # accumulates via identity matmuls into psum seeded with
                # -Hsum (fp16 residual pair, no data deps).
                ps_cx = psB.tile([BC, D], f32, tag="misc")
                nc.tensor.matmul(ps_cx, ident, negA, start=True, stop=False)
                nc.tensor.matmul(ps_cx, ident, negB, start=False, stop=False)
                TS = T - POOL_T
                ah_pool = work.tile([BC, POOL_T, D], f16, tag="ahpool")
                hb = h_nat[:, TS:, :]
                al = rsd[:, TS:]
                nc.gpsimd.tensor_tensor(
                    out=ah_pool, in0=hb,
                    in1=bass.AP(tensor=al.tensor, offset=al.offset,
                                ap=[al.ap[0], [al.ap[-1][0], POOL_T],
                                    [0, D]]),
                    op=ALU.mult)
                ahs = []
                for t in range(TS):
                    ah = ahp.tile([BC, D], f16, tag="ah")
                    if t % 5 == 4:
                        nc.scalar.activation(
                            out=ah, in_=h_nat[:, t, :], func=AF.Copy,
                            scale=rsd[:, t:t + 1])
                    else:
                        nc.vector.tensor_scalar(
                            out=ah, in0=h_nat[:, t, :],
                            scalar1=rsd[:, t:t + 1],
                            scalar2=None, op0=ALU.mult)
                    ahs.append(ah)
                    if t % 16 == 15 or t == TS - 1:
                        for a in ahs:
                            nc.tensor.matmul(ps_cx, ident, a, start=False,
                                             stop=False)
                        ahs = []
                assert not ahs
                for j in range(POOL_T):
                    nc.tensor.matmul(ps_cx, ident, ah_pool[:, j, :],
                                     start=False, stop=(j == POOL_T - 1))
                # 1/Z rescale on ACT (Copy takes a per-partition AP scale
                # and reads psum through the faster port)
                ctxv = small.tile([BC, D], f16, tag="ctxv")
                nc.scalar.activation(out=ctxv, in_=ps_cx, func=AF.Copy,
                                     scale=rzb[:, 0:1])
                # transpose context -> [d', m, b]
                ps_ct = psB.tile([BC, KT, 128], f16, tag="misc")
                ctxT = small.tile([128, KT, 128], f16, tag="ctxT")
                for m in range(KT):
                    nc.tensor.transpose(
                        ps_ct[:, m, :], ctxv[:, m * 128:(m + 1) * 128],
                        ident)
                    nc.vector.tensor_copy(ctxT[:, m, :], ps_ct[:, m, :])

                # per-half tail: gates -> LSTM -> hp. Emission order keeps
                # each engine FIFO clean: gatesA, gatesB (PE) / lstmA,
                # lstmB (ACT+DVE) / hpA, hpB (PE) with the hpR drains last,
                # so next step's half-A z stream starts while half B
                # finishes. Gate psum is (half, mg, b')-major: each half
                # owns disjoint banks, so B's accumulation never collides
                # with A's LSTM reads.
                ps_g = psA.tile([128, 2, GT, 64], f32, tag="eB")
                for hi in (0, 1):
                    hsl = slice(hi * 64, hi * 64 + 64)
                    for mg in range(GT):
                        gsl = slice(mg * 128, (mg + 1) * 128)
                        nc.tensor.matmul(
                            ps_g[:, hi, mg, :], ident, ghosb[:, mg, hsl],
                            start=True, stop=False)
                        for k in range(KT):
                            nc.tensor.matmul(
                                ps_g[:, hi, mg, :], wxc[:, k, gsl],
                                ctxT[:, k, hsl],
                                start=False, stop=(k == KT - 1))
                for hi in (0, 1):
                    hsl = slice(hi * 64, hi * 64 + 64)
                    # LSTM pointwise, transposed [g', b']. permuted gate
                    # order i,f,o,g -> m-tiles 0:2 = i, 2:4 = f, 4:6 = o,
                    # 6:8 = g (b_lstm folded into wxo host-side).
                    sig_ifo = small.tile([128, 6, 64], f16, tag="sig_ifo")
                    tg = small.tile([128, KT, 64], f16, tag="tg")
                    nc.scalar.activation(out=sig_ifo,
                                         in_=ps_g[:, hi, 0:6, :],
                                         func=AF.Sigmoid)
                    nc.scalar.activation(out=tg, in_=ps_g[:, hi, 6:8, :],
                                         func=AF.Tanh)
                    t1 = small.tile([128, KT, 64], f16, tag="t1")
                    t2 = small.tile([128, KT, 64], f16, tag="t2")
                    nc.vector.tensor_tensor(out=t1, in0=sig_ifo[:, 2:4, :],
                                            in1=cT[:, :, hsl], op=ALU.mult)
                    nc.vector.tensor_tensor(out=t2, in0=sig_ifo[:, 0:2, :],
                                            in1=tg, op=ALU.mult)
                    nc.vector.tensor_tensor(out=cT[:, :, hsl], in0=t1,
                                            in1=t2, op=ALU.add)
                    tc_t = small.tile([128, KT, 64], f16, tag="tc_t")
                    nc.scalar.activation(out=tc_t, in_=cT[:, :, hsl],
                                         func=AF.Tanh)
                    # h goes straight into hidT layout - no transpose
                    nc.vector.tensor_tensor(out=hidT[:, :, s + 1, hsl],
                                            in0=sig_ifo[:, 4:6, :],
                                            in1=tc_t, op=ALU.mult)
                    if s < S - 1:
                        emit_hp_half(s + 1, hi)

            emit_gen(S - 1)

    _split_excess_waits(nc)
    return nc


def _get_module():
    if "nc" not in _CACHE:
        _CACHE["nc"] = _build()
    return _CACHE["nc"]


def build_in_maps(batch_H, text, batch_max_length, Wi2h, Wh2h, bh2h, w_score,
                  Wx, Wh, b_lstm, Wg, bg):
    batch_H = np.asarray(batch_H, dtype=np.float32)
    text = np.asarray(text)
    assert int(batch_max_length) + 1 == S
    assert batch_H.shape == (B, T, D)

    f16 = np.float16
    bh16 = batch_H.astype(f16)
    # one-hot text: [B, S, C] -> per-core [S, C, BC]
    oh = (text[:, :S, None] == np.arange(C)[None, None, :])

    # permute keras gate order (i, f, c, o) -> (i, f, o, c)
    perm = np.concatenate([np.arange(0, 2 * H),          # i, f
                           np.arange(3 * H, 4 * H),      # o
                           np.arange(2 * H, 3 * H)])     # c/g
    Wx = np.asarray(Wx, np.float32)[:, perm]
    Wh_p = np.asarray(Wh, np.float32)[:, perm]
    bl_p = np.asarray(b_lstm, np.float32)[perm]
    wxo_p = (Wx[D:D + C, :] + bl_p[None, :]).astype(f16)
    weights = {
        "wi2h": np.ascontiguousarray(np.asarray(Wi2h, np.float32).astype(f16)),
        "wh2h": np.ascontiguousarray(np.asarray(Wh2h, np.float32).astype(f16)),
        "bh2hT": np.ascontiguousarray(
            np.asarray(bh2h, np.float32).reshape(H, 1)),
        "wsc": np.ascontiguousarray(np.tile(
            np.asarray(w_score, np.float32).reshape(H, 1), (1, 128)).astype(f16)),
        "wxc": np.ascontiguousarray(Wx[:D, :].astype(f16)),
        "wxo": np.ascontiguousarray(wxo_p),
        "wh": np.ascontiguousarray(Wh_p.astype(f16)),
        "wg": np.ascontiguousarray(np.asarray(Wg, np.float32).astype(f16)),
    }

    in_maps = []
    for c in range(NCORES):
        bsl = slice(c * BC, (c + 1) * BC)
        in_maps.append({
            "h_nat": np.ascontiguousarray(bh16[bsl]),
            "h_t": np.ascontiguousarray(bh16[bsl].transpose(2, 0, 1)),
            "onehot": np.ascontiguousarray(
                oh[bsl].transpose(2, 1, 0).astype(f16)),
            **weights,
        })
    return in_maps


def kernel(**inputs):
    in_maps = build_in_maps(**inputs)
    bg = inputs["bg"]

    nc = _get_module()
    res = run_bass_kernel_spmd(nc, in_maps, list(range(NCORES)))

    out = np.empty((B, S, C), np.float32)
    for c in range(NCORES):
        out[c * BC:(c + 1) * BC] = res.results[c]["probsT"].transpose(2, 1, 0)
    out += np.asarray(bg, np.float32)[None, None, :]
    return out


if __name__ == "__main__":
    _build()
    print("build OK")


# revision 54
# speedup vs baseline: 1.2272x; 1.0925x over previous
"""Trainium2 Bass kernel for an attention-LSTM decoder (Bahdanau attention +
LSTM cell + generator head), data-parallel over 8 NeuronCores.

Shapes (hardcoded): B=1024, T=64, D=256, H=256, C=96, steps S=26.
Each core handles 128 batch rows.

Device layouts (per core, b = 128):
  - score chain runs "transposed": partitions = hidden dim tiles (2 x 128),
    free = (b, t) flat b-major.
  - softmax/context run natural: partitions = b, free = t / d.
  - gates + LSTM pointwise run TRANSPOSED (partitions = gate-dim tiles,
    free = b) so h lands directly in hidT layout - no h transpose.
Matmul operands are fp16 (full-rate PE streaming); PSUM accumulation is
fp32; the recurrent c state is fp16 (transposed).

The recurrence is independent per batch row, so each step is split into
two b-halves that pipeline through the engines: half A's gates -> LSTM ->
hp chain finishes first and its z(s+1) stream starts while half B is
still in its step-s tail. Only the alpha*H chain and context need full
width. Per-step pipeline (per half): z = projT + hp (DVE, bcast AP; bh2h
pre-folded into projT; hp drained from psum directly in 2x-replicated
form) -> tanh (ACT) -> e-matmul (M=128-replicated w_score) into 2-bank
psum waves -> drain fp16 -> DMA-scatter to [b, t]. The (96,128) tail
gates the whole next phase, so chunks 12-15 drain first, the last wave
borrows the misc psum slot (free during the z phase) and keeps drain +
DMA issue + sigmoid on one ACT FIFO. Softmax r = 1/sigmoid(-e) = alpha+1
(resident sigmoid table, no exp switch, no normalize op): the context
psum is seeded with -sum_t H_t (fp16 residual pair) so unnormalized r
works directly, and 1/Z = 1/(sum r - T) is computed off-path. alpha*H
runs full-width split across DVE (4x tensor_scalar) / ACT (scale AP) /
Pool (one fused op), PE identity-accumulates into the seeded psum.
Gates + LSTM pointwise run per half in the transposed layout with one
fused sigmoid over i|f|o; gate psum is (half, mg, b')-major so each half
owns disjoint banks. Generator matmuls are emitted at the top of the
next step so the PE queue reaches them while stalled on alpha.

Host-side prep (numpy): fp16 casts, batch_H transpose for the projection
matmul, one-hot text encoding, gate columns permuted keras (i,f,c,o) ->
(i,f,o,c), b_lstm folded into the one-hot weight rows (valid because
one-hot rows sum to 1), bg added to the final output.
"""

import sys

for _p in ("/opt/trn_rl_repo", "/root/.axon_site/_ro/trn_rl_repo"):
    if _p not in sys.path:
        sys.path.insert(0, _p)

import numpy as np

import concourse.bass as bass
import concourse.tile as tile
from concourse import mybir
from concourse.bass_utils import run_bass_kernel_spmd
from concourse.masks import make_identity

dt = mybir.dt
AF = mybir.ActivationFunctionType
ALU = mybir.AluOpType

NCORES = 8
B, T, D, H, C = 1024, 64, 256, 256, 96
S = 26  # num steps = batch_max_length + 1
BC = B // NCORES  # 128 batch rows per core
KT = 2  # 256 = 2 x 128 tiles for d/h contraction
GT = 8  # 4H = 1024 = 8 m-tiles of 128
TB = BC * T  # 8192, flat (b, t) b-major
NCHUNK = 512  # psum-bank-limited matmul N
EWAVE = 2  # e-matmul psum slots are 2 banks (eB/eA2); the transposed
# gates psums multiplex onto eB (all uses sequential within a step),
# single-chunk waves ride the 1-bank "gen" slot, and the last 2-chunk
# wave borrows the misc slot (free during the z phase): 2+2+2+1 banks.
POOL_T = 8  # trailing t's of the alpha*H chain handled by one Pool op
# (Pool Multiply runs at 0.42 efficiency: 8 t's ~ 4.2us; Pool does no
# other per-step work since it cannot touch PSUM)

_CACHE = {}


def _split_excess_waits(nc, max_waits=1):
    """This container's walrus rejects instructions carrying more than
    ~max_waits semaphore waits ("Too many sync wait commands"). Hoist excess
    waits onto InstNoOp instructions inserted just before, on the same engine
    (per-engine program order makes this semantics-preserving)."""
    nid = [0]
    for f in nc.m.functions:
        for blk in f.blocks:
            insts = blk.instructions
            out = []
            changed = False
            for ins in insts:
                si = ins.sync_info
                ow = list(si.on_wait) if si is not None and si.on_wait else []
                if len(ow) > max_waits:
                    changed = True
                    while len(ow) > max_waits:
                        take, ow = ow[:max_waits], ow[max_waits:]
                        nid[0] += 1
                        nop = mybir.InstNoOp(
                            name=f"WSPLIT-{nid[0]}", engine=ins.engine,
                            sync_info=mybir.SyncInfo(on_wait=take,
                                                     on_update=[]))
                        nc.register_instruction(nop, overwrite=True)
                        out.append(nop)
                    ins.sync_info = mybir.SyncInfo(
                        on_wait=ow, on_update=list(si.on_update or []))
                out.append(ins)
            if changed:
                blk.instructions = out


def _build():
    nc = bass.Bass("TRN2", target_bir_lowering=False)
    f16, f32 = dt.float16, dt.float32

    h_nat_d = nc.declare_dram_parameter("h_nat", [BC, T, D], f16, isOutput=False)
    h_t_d = nc.declare_dram_parameter("h_t", [D, BC, T], f16, isOutput=False)
    oneh_d = nc.declare_dram_parameter("onehot", [C, S, BC], f16, isOutput=False)
    wi2h_d = nc.declare_dram_parameter("wi2h", [D, H], f16, isOutput=False)
    wh2h_d = nc.declare_dram_parameter("wh2h", [H, H], f16, isOutput=False)
    bh2h_d = nc.declare_dram_parameter("bh2hT", [H, 1], f32, isOutput=False)
    wsc_d = nc.declare_dram_parameter("wsc", [H, 128], f16, isOutput=False)
    wxc_d = nc.declare_dram_parameter("wxc", [D, 4 * H], f16, isOutput=False)
    wxo_d = nc.declare_dram_parameter("wxo", [C, 4 * H], f16, isOutput=False)
    wh_d = nc.declare_dram_parameter("wh", [H, 4 * H], f16, isOutput=False)
    wg_d = nc.declare_dram_parameter("wg", [H, C], f16, isOutput=False)
    probs_d = nc.declare_dram_parameter("probsT", [C, S, BC], f32, isOutput=True)

    with tile.TileContext(nc) as tc:
        import contextlib
        ctx = contextlib.ExitStack()
        with ctx:
            singles = ctx.enter_context(tc.tile_pool(name="singles", bufs=1))
            # psA: two 2-bank e-matmul wave slots (eB/eA2) so the PE can
            # fill one while the other drains; the transposed gates psums
            # multiplex onto eB (all uses sequential within a step).
            psA = ctx.enter_context(tc.tile_pool(name="psA", bufs=1, space="PSUM"))
            psB = ctx.enter_context(tc.tile_pool(name="psB", bufs=1, space="PSUM"))

            # ---- persistent SBUF state ----
            h_nat = singles.tile([BC, T, D], f16)
            projT = singles.tile([128, KT, BC, T], f16)  # [h', m, b, t]
            hidT = singles.tile([128, KT, S + 1, BC], f16)  # h states, slot 0 = 0
            oneh = singles.tile([C, S, BC], f16)
            wi2h = singles.tile([128, KT, H], f16)
            wh2h = singles.tile([128, KT, H], f16)
            bh2hT = singles.tile([128, KT, 1], f32)
            wsc = singles.tile([128, KT, 128], f16)
            wxc = singles.tile([128, KT, 4 * H], f16)
            wxo = singles.tile([C, 4 * H], f16)
            wh = singles.tile([128, KT, 4 * H], f16)
            wg = singles.tile([128, KT, C], f16)
            ident = singles.tile([128, 128], f16)
            cT = singles.tile([128, KT, BC], f16)
            # 4-slot ring: each step's probs are DMA'd out immediately
            probs_sb = singles.tile([C, 4, BC], f32)

            # ---- load everything ----
            # (h_nat is issued after the h_tt chunks below: it is not read
            # until step 0's context phase, but 4 MB at the front of the
            # sync queue would delay the proj-critical h_tt transfer)
            nc.sync.dma_start(out=oneh, in_=oneh_d[:])
            nc.sync.dma_start(
                out=wi2h, in_=wi2h_d[:].rearrange("(k p) h -> p k h", p=128))
            nc.sync.dma_start(
                out=wh2h, in_=wh2h_d[:].rearrange("(k p) h -> p k h", p=128))
            nc.sync.dma_start(
                out=bh2hT, in_=bh2h_d[:].rearrange("(k p) o -> p k o", p=128))
            nc.sync.dma_start(
                out=wsc, in_=wsc_d[:].rearrange("(k p) o -> p k o", p=128))
            nc.sync.dma_start(
                out=wxc, in_=wxc_d[:].rearrange("(k p) g -> p k g", p=128))
            nc.sync.dma_start(out=wxo, in_=wxo_d[:])
            nc.sync.dma_start(
                out=wh, in_=wh_d[:].rearrange("(k p) g -> p k g", p=128))
            nc.sync.dma_start(
                out=wg, in_=wg_d[:].rearrange("(k p) c -> p k c", p=128))
            make_identity(nc, ident)
            nc.vector.memset(hidT[:, :, 0, :], 0.0)
            nc.vector.memset(cT, 0.0)

            # ---- precompute projT = (batch_H @ Wi2h)^T + bh2h : [h',m,(b t)]
            projT_f = projT[:].rearrange("p m b t -> p m (b t)")
            with tc.tile_pool(name="ht", bufs=1) as ht_pool:
                h_tt = ht_pool.tile([128, KT, BC, T], f16)
                # load in 4 b-chunks so the first proj matmuls start after a
                # quarter of the transfer instead of all of it
                for lb in range(4):
                    bls = slice(lb * (BC // 4), (lb + 1) * (BC // 4))
                    nc.sync.dma_start(
                        out=h_tt[:, :, bls, :],
                        in_=h_t_d[:, bls, :].rearrange(
                            "(k p) b t -> p k b t", p=128))
                nc.sync.dma_start(out=h_nat, in_=h_nat_d[:])
                h_tt_f = h_tt[:].rearrange("p k b t -> p k (b t)")
                nchk = TB // NCHUNK
                ncw = (nchk + EWAVE - 1) // EWAVE
                for m in range(KT):
                    for w in range(ncw):
                        nb = min(EWAVE, nchk - w * EWAVE)
                        ps = psA.tile([128, EWAVE, NCHUNK], f32,
                                      tag=("eB" if w % 2 == 0 else "eA2"))
                        for j in range(nb):
                            sl = slice((w * EWAVE + j) * NCHUNK,
                                       (w * EWAVE + j + 1) * NCHUNK)
                            for k in range(KT):
                                nc.tensor.matmul(
                                    ps[:, j, :],
                                    wi2h[:, k, m * 128:(m + 1) * 128],
                                    h_tt_f[:, k, sl], start=(k == 0),
                                    stop=(k == KT - 1))
                        sl3 = slice(w * EWAVE * NCHUNK,
                                    (w * EWAVE + nb) * NCHUNK)
                        pin = ps[:, :nb, :].rearrange("p a n -> p (a n)")
                        # fold the bh2h bias in during the drain, DVE/ACT
                        # alternating (Identity, unlike Copy, takes an AP
                        # bias; Pool can't read psum) - preamble only
                        if w % 2 == 0:
                            nc.vector.tensor_scalar(
                                out=projT_f[:, m, sl3], in0=pin,
                                scalar1=bh2hT[:, m, :], scalar2=None,
                                op0=ALU.add)
                        else:
                            nc.scalar.activation(
                                out=projT_f[:, m, sl3], in_=pin,
                                func=AF.Identity, bias=bh2hT[:, m, :])

            work = ctx.enter_context(tc.tile_pool(name="work", bufs=2))
            small = ctx.enter_context(tc.tile_pool(name="small", bufs=2))
            ahp = ctx.enter_context(tc.tile_pool(name="ahp", bufs=16))

            # ---- Hsum = sum_t H[:, t, :] (for the unnormalized-alpha
            # context trick: sum_t r_t H_t - Hsum = sum_t alpha_t H_t with
            # r = 1/sig(-e) = alpha + 1). Stored negated as an fp16
            # residual pair so the psum pre-load is fp32-exact.
            negA = singles.tile([BC, D], f16)
            negB = singles.tile([BC, D], f16)
            # dedicated drain buffer for the final e chunk: the rolling esb
            # pool would serialize the tail behind older waves' drains/DMAs
            esb15 = singles.tile([128, NCHUNK], f16)
            ps_hs = psB.tile([BC, D], dt.float32, tag="misc")
            for t in range(T):
                nc.tensor.matmul(ps_hs, ident, h_nat[:, t, :],
                                 start=(t == 0), stop=(t == T - 1))
            nc.scalar.activation(out=negA, in_=ps_hs, func=AF.Copy,
                                 scale=-1.0)
            hs32 = singles.tile([BC, D], dt.float32)
            nc.vector.tensor_scalar(out=hs32, in0=ps_hs, scalar1=-1.0,
                                    scalar2=None, op0=ALU.mult)
            nc.vector.tensor_tensor(out=negB, in0=hs32, in1=negA,
                                    op=ALU.subtract)

            # ---- decode steps ----
            def emit_gen(s):
                # generator matmuls for step s (reads hidT slot s+1); emitted
                # at the top of step s+1 so the PE queue reaches them while
                # stalled on step s+1's alpha (fills the tail gap). The
                # per-step output DMA hides the 1.3 MB store under compute.
                ps_p = psB.tile([C, 128], f32, tag="gen")
                for k in range(KT):
                    nc.tensor.matmul(
                        ps_p, wg[:, k, :], hidT[:, k, s + 1, :],
                        start=(k == 0), stop=(k == KT - 1))
                sl = s % 4
                nc.scalar.copy(out=probs_sb[:, sl, :], in_=ps_p)
                nc.sync.dma_start(out=probs_d[:, s, :],
                                  in_=probs_sb[:, sl, :])

            # ---- half-pipelined decode ----
            # The recurrence is independent per batch row, and in both the
            # score layout (partitions = h') and the transposed gate/LSTM
            # layout (partitions = g') the batch lives on the free axis, so
            # splitting b into halves halves those ops' cost. Half A's
            # gates->LSTM->hp chain finishes first and its z(s+1) stream
            # starts while half B is still in its step-s tail; only the
            # full-width phases (alpha*H chain, ctx) need both halves.
            esbT = [singles.tile([128, 2, NCHUNK], f16, name=f"esbT{i}")
                    for i in range(4)]
            hpRs = {}

            def emit_hp_half(s, hi):
                # hp = h @ Wh2h for one b-half, transposed [h', m, b'];
                # drained straight into the 2x-replicated form the z-add's
                # broadcast AP wants (last dim step-1)
                hsl = slice(hi * 64, hi * 64 + 64)
                ps_hp = psB.tile([128, KT, 64], f32, tag="misc")
                for m in range(KT):
                    for k in range(KT):
                        nc.tensor.matmul(
                            ps_hp[:, m, :],
                            wh2h[:, k, m * 128:(m + 1) * 128],
                            hidT[:, k, s, hsl], start=(k == 0),
                            stop=(k == KT - 1))
                hpR = small.tile([128, KT, 64, 2], f16, tag=f"hpR{hi}")
                for m in range(KT):
                    base = ps_hp[:, m, :]
                    nc.vector.tensor_copy(
                        hpR[:, m, :, :],
                        bass.AP(tensor=base.tensor, offset=base.offset,
                                ap=[base.ap[0], [base.ap[-1][0], 64],
                                    [0, 2]]))
                hpRs[hi] = hpR

            for hi in (0, 1):
                emit_hp_half(0, hi)

            for s in range(S):
                if s > 0:
                    emit_gen(s - 1)

                alpha_e = small.tile([BC, T], f16, tag="alphae")
                sden = small.tile([BC, T], f16, tag="sden")
                rsd = small.tile([BC, T], f32, tag="rsd")

                def emit_softmax(h0, h1):
                    # r = 1/sigmoid(-e) = exp(e) + 1: unnormalized alpha
                    # offset by +1; the context psum pre-subtracts Hsum to
                    # compensate. sig(-e) is exact via the resident sigmoid
                    # table; no exp table switch.
                    nc.scalar.activation(out=sden[h0:h1, :],
                                         in_=alpha_e[h0:h1, :],
                                         func=AF.Sigmoid, scale=-1.0)
                    with nc.allow_low_precision(
                            reason="sig(-e) in [0.05, 0.95]; fp16 adds "
                                   "~5e-4 rel to alpha, under the 2e-2 "
                                   "budget"):
                        nc.vector.reciprocal(out=rsd[h0:h1, :],
                                             in_=sden[h0:h1, :])

                # z = projT + hp (bcast over t) per half; tanh on ACT;
                # e = w . tanh per 512-col chunk into psum waves, drained
                # fp16 and DMA-scattered to [b, t]. Single-chunk waves ride
                # dedicated buffers + the 1-bank gen slot so the per-half
                # tails never wait on rolling resources.
                ths = {}

                def z_groups(hi, groups, b0, upto=None):
                    base_b = hi * 64
                    hpR = hpRs[hi]
                    for nbz in groups:
                        bsl = slice(b0, b0 + nbz)
                        # both m-tiles of a group share one z buffer so a
                        # single tanh covers them (halves the ACT per-op
                        # SBUF-access overhead in the z-bound phase)
                        pool_g = hi == 1 and nbz in (32, 24)
                        z = work.tile([128, KT, 32, T], f16,
                                      tag="zp" if pool_g else "z")
                        for m in range(KT):
                            # half B's big-group m0 adds ride the idle
                            # Pool engine: their tanh slots come after the
                            # whole tanh-A stream, so Pool's 3.3x-slower
                            # add is fully hidden. Those groups get their
                            # own buffer tag so the long hold doesn't
                            # stall the z ring.
                            hb = hpR[:, m, b0 - base_b:, :]
                            eng = (nc.gpsimd if (pool_g and m == 0)
                                   else nc.vector)
                            eng.tensor_tensor(
                                out=z[:, m, :nbz, :].rearrange(
                                    "p b (r i) -> p b r i", i=2),
                                in0=projT[:, m, bsl, :].rearrange(
                                    "p b (r i) -> p b r i", i=2),
                                in1=bass.AP(
                                    tensor=hb.tensor, offset=hb.offset,
                                    ap=[hb.ap[0], [hb.ap[1][0], nbz],
                                        [0, T // 2], [1, 2]]),
                                op=ALU.add)
                        th = work.tile([128, KT, 32 * T], f16,
                                       tag=f"th{hi}")
                        nc.scalar.activation(
                            out=th[:, :, :nbz * T],
                            in_=z[:, :, :nbz, :].rearrange(
                                "p m b t -> p m (b t)"),
                            func=AF.Tanh)
                        for q in range(nbz * T // NCHUNK):
                            c = (b0 - base_b) // 8 + q + hi * 8
                            ths[c] = (th, q * NCHUNK)
                        b0 += nbz
                    return b0

                def e_mms(chunks, tag):
                    nb = len(chunks)
                    if tag == "gen":
                        ps_e = psB.tile([128, 1, NCHUNK], f32, tag="gen")
                    elif tag == "misc":
                        ps_e = psB.tile([128, 2, NCHUNK], f32, tag="misc")
                    else:
                        ps_e = psA.tile([128, EWAVE, NCHUNK], f32, tag=tag)
                    for j, c in enumerate(chunks):
                        th, off = ths[c]
                        for m in range(KT):
                            nc.tensor.matmul(
                                ps_e[:, j, :], wsc[:, m, :],
                                th[:, m, off:off + NCHUNK],
                                start=(m == 0), stop=(m == KT - 1))
                    return ps_e

                def e_out(chunks, ps_e, ded, deng, dma_eng=None):
                    nb = len(chunks)
                    if ded is not None:
                        ebuf = esbT[ded]
                        tgt = ebuf[:, :nb, :]
                    else:
                        ebuf = work.tile([128, EWAVE, NCHUNK], f16,
                                         tag="esb")
                        tgt = ebuf[:, :nb, :]
                    if deng == "act":
                        nc.scalar.copy(out=tgt, in_=ps_e[:, :nb, :])
                    else:
                        nc.vector.tensor_copy(tgt, ps_e[:, :nb, :])
                    eb = ebuf[0:1, 0:nb, :]
                    (dma_eng or nc.sync).dma_start(
                        out=alpha_e[chunks[0] * 8:
                                    chunks[0] * 8 + nb * 8, :],
                        in_=bass.AP(tensor=eb.tensor, offset=eb.offset,
                                    ap=[eb.ap[0], [1, nb * NCHUNK]]),
                        single_packet=True)

                # --- half A: z stream + waves, inline drains (its tail is
                # hidden under half B's z stream) ---
                b0 = z_groups(0, (8, 24, 32), 0)
                assert b0 == 64
                for chunks, tag, ded, deng in (
                        ((0,), "gen", 0, "dve"),
                        ((1, 2), "eB", None, "dve"),
                        ((3, 4), "eA2", None, "dve"),
                        ((5, 6), "eB", None, "dve"),
                        ((7,), "gen", 1, "act")):
                    ps_e = e_mms(chunks, tag)
                    e_out(chunks, ps_e, ded, deng)
                emit_softmax(0, 64)

                # --- half B: the (96,128) tail gates the whole ah phase,
                # so chunks 12-15 drain first; the last wave rides the
                # misc psum slot (free during the z phase) and its drain +
                # DMA issue + sigmoid all stay on the ACT FIFO ---
                b0 = z_groups(1, (32, 24), 64)
                ps8 = e_mms((8,), "gen")
                e_out((8,), ps8, 2, "dve")
                ps910 = e_mms((9, 10), "eA2")
                ps11 = e_mms((11,), "gen")
                b0 = z_groups(1, (8,), b0)
                assert b0 == BC
                ps1213 = e_mms((12, 13), "eB")
                ps1415 = e_mms((14, 15), "misc")
                e_out((12, 13), ps1213, None, "dve")
                e_out((14, 15), ps1415, 3, "act", dma_eng=nc.scalar)
                emit_softmax(96, BC)
                e_out((9, 10), ps910, None, "dve")
                e_out((11,), ps11, 0, "dve")
                emit_softmax(64, 96)

                # gates partial (h and onehot contributions don't need ctx);
                # transposed: out [g', b], full width. No data deps, so the
                # PE chews it during the softmax tail, keeping the pstate
                # ramp alive for the ident accumulates. Each m-tile's psum
                # group is closed immediately (one open accumulation group
                # per psum bank) and drained to SBUF; re-injected per half.
                ps_gp = psA.tile([128, GT, 128], f32, tag="eB")
                for mg in range(GT):
                    gsl = slice(mg * 128, (mg + 1) * 128)
                    for k in range(KT):
                        nc.tensor.matmul(
                            ps_gp[:, mg, :], wh[:, k, gsl],
                            hidT[:, k, s, :], start=(k == 0), stop=False)
                    nc.tensor.matmul(
                        ps_gp[:, mg, :], wxo[:, gsl], oneh[:, s, :],
                        start=False, stop=True)
                ghosb = work.tile([128, GT, 128], f16, tag="gho")
                nc.vector.tensor_copy(ghosb, ps_gp)

                # 1/Z for the later ctx rescale: Z = sum_t r - T, off the
                # critical path (only needed once the ah chain finishes)
                zs = small.tile([BC, 1], f32, tag="zsum")
                nc.vector.reduce_sum(zs, rsd, mybir.AxisListType.X)
                zm = small.tile([BC, 1], f32, tag="zm")
                nc.vector.tensor_scalar(out=zm, in0=zs, scalar1=-float(T),
                                        scalar2=None, op0=ALU.add)
                rzb = small.tile([BC, 1], f32, tag="rzb")
                nc.vector.reciprocal(out=rzb, in_=zm)

                # context = sum_t r[:, t] * H[:, t, :] - Hsum (r = alpha+1),
                # full width: r_t*H_t slices at 4x on DVE with ACT every 5th
                # t and Pool one fused op for the last POOL_T t's; PE
                # accumulates via identity matmuls into psum seeded with
                # -Hsum (fp16 residual pair, no data deps).
                ps_cx = psB.tile([BC, D], f32, tag="misc")
                nc.tensor.matmul(ps_cx, ident, negA, start=True, stop=False)
                nc.tensor.matmul(ps_cx, ident, negB, start=False, stop=False)
                TS = T - POOL_T
                ah_pool = work.tile([BC, POOL_T, D], f16, tag="ahpool")
                hb = h_nat[:, TS:, :]
                al = rsd[:, TS:]
                nc.gpsimd.tensor_tensor(
                    out=ah_pool, in0=hb,
                    in1=bass.AP(tensor=al.tensor, offset=al.offset,
                                ap=[al.ap[0], [al.ap[-1][0], POOL_T],
                                    [0, D]]),
                    op=ALU.mult)
                ahs = []
                for t in range(TS):
                    ah = ahp.tile([BC, D], f16, tag="ah")
                    if t % 5 == 4:
                        nc.scalar.activation(
                            out=ah, in_=h_nat[:, t, :], func=AF.Copy,
                            scale=rsd[:, t:t + 1])
                    else:
                        nc.vector.tensor_scalar(
                            out=ah, in0=h_nat[:, t, :],
                            scalar1=rsd[:, t:t + 1],
                            scalar2=None, op0=ALU.mult)
                    ahs.append(ah)
                    if t % 16 == 15 or t == TS - 1:
                        for a in ahs:
                            nc.tensor.matmul(ps_cx, ident, a, start=False,
                                             stop=False)
                        ahs = []
                assert not ahs
                for j in range(POOL_T):
                    nc.tensor.matmul(ps_cx, ident, ah_pool[:, j, :],
                                     start=False, stop=(j == POOL_T - 1))
                # 1/Z rescale on ACT (Copy takes a per-partition AP scale
                # and reads psum through the faster port)
                ctxv = small.tile([BC, D], f16, tag="ctxv")
                nc.scalar.activation(out=ctxv, in_=ps_cx, func=AF.Copy,
                                     scale=rzb[:, 0:1])
                # transpose context -> [d', m, b]
                ps_ct = psB.tile([BC, KT, 128], f16, tag="misc")
                ctxT = small.tile([128, KT, 128], f16, tag="ctxT")
                for m in range(KT):
                    nc.tensor.transpose(
                        ps_ct[:, m, :], ctxv[:, m * 128:(m + 1) * 128],
                        ident)
                    nc.vector.tensor_copy(ctxT[:, m, :], ps_ct[:, m, :])

                # per-half tail: gates -> LSTM -> hp. Emission order keeps
                # each engine FIFO clean: gatesA, gatesB (PE) / lstmA,
                # lstmB (ACT+DVE) / hpA, hpB (PE) with the hpR drains last,
                # so next step's half-A z stream starts while half B
                # finishes. Gate psum is (half, mg, b')-major: each half
                # owns disjoint banks, so B's accumulation never collides
                # with A's LSTM reads.
                ps_g = psA.tile([128, 2, GT, 64], f32, tag="eB")
                for hi in (0, 1):
                    hsl = slice(hi * 64, hi * 64 + 64)
                    for mg in range(GT):
                        gsl = slice(mg * 128, (mg + 1) * 128)
                        nc.tensor.matmul(
                            ps_g[:, hi, mg, :], ident, ghosb[:, mg, hsl],
                            start=True, stop=False)
                        for k in range(KT):
                            nc.tensor.matmul(
                                ps_g[:, hi, mg, :], wxc[:, k, gsl],
                                ctxT[:, k, hsl],
                                start=False, stop=(k == KT - 1))
                for hi in (0, 1):
                    hsl = slice(hi * 64, hi * 64 + 64)
                    # LSTM pointwise, transposed [g', b']. permuted gate
                    # order i,f,o,g -> m-tiles 0:2 = i, 2:4 = f, 4:6 = o,
                    # 6:8 = g (b_lstm folded into wxo host-side).
                    sig_ifo = small.tile([128, 6, 64], f16, tag="sig_ifo")
                    tg = small.tile([128, KT, 64], f16, tag="tg")
                    nc.scalar.activation(out=sig_ifo,
                                         in_=ps_g[:, hi, 0:6, :],
                                         func=AF.Sigmoid)
                    nc.scalar.activation(out=tg, in_=ps_g[:, hi, 6:8, :],
                                         func=AF.Tanh)
                    t1 = small.tile([128, KT, 64], f16, tag="t1")
                    t2 = small.tile([128, KT, 64], f16, tag="t2")
                    nc.vector.tensor_tensor(out=t1, in0=sig_ifo[:, 2:4, :],
                                            in1=cT[:, :, hsl], op=ALU.mult)
                    nc.vector.tensor_tensor(out=t2, in0=sig_ifo[:, 0:2, :],
                                            in1=tg, op=ALU.mult)
                    nc.vector.tensor_tensor(out=cT[:, :, hsl], in0=t1,
                                            in1=t2, op=ALU.add)
                    tc_t = small.tile([128, KT, 64], f16, tag="tc_t")
                    nc.scalar.activation(out=tc_t, in_=cT[:, :, hsl],
                                         func=AF.Tanh)
                    # h goes straight into hidT layout - no transpose
                    nc.vector.tensor_tensor(out=hidT[:, :, s + 1, hsl],
                                            in0=sig_ifo[:, 4:6, :],
                                            in1=tc_t, op=ALU.mult)
                    if s < S - 1:
                        emit_hp_half(s + 1, hi)

            emit_gen(S - 1)

    _split_excess_waits(nc)
    return nc


def _get_module():
    if "nc" not in _CACHE:
        _CACHE["nc"] = _build()
    return _CACHE["nc"]


def build_in_maps(batch_H, text, batch_max_length, Wi2h, Wh2h, bh2h, w_score,
                  Wx, Wh, b_lstm, Wg, bg):
    batch_H = np.asarray(batch_H, dtype=np.float32)
    text = np.asarray(text)
    assert int(batch_max_length) + 1 == S
    assert batch_H.shape == (B, T, D)

    f16 = np.float16
    bh16 = batch_H.astype(f16)
    # one-hot text: [B, S, C] -> per-core [S, C, BC]
    oh = (text[:, :S, None] == np.arange(C)[None, None, :])

    # permute keras gate order (i, f, c, o) -> (i, f, o, c)
    perm = np.concatenate([np.arange(0, 2 * H),          # i, f
                           np.arange(3 * H, 4 * H),      # o
                           np.arange(2 * H, 3 * H)])     # c/g
    Wx = np.asarray(Wx, np.float32)[:, perm]
    Wh_p = np.asarray(Wh, np.float32)[:, perm]
    bl_p = np.asarray(b_lstm, np.float32)[perm]
    wxo_p = (Wx[D:D + C, :] + bl_p[None, :]).astype(f16)
    weights = {
        "wi2h": np.ascontiguousarray(np.asarray(Wi2h, np.float32).astype(f16)),
        "wh2h": np.ascontiguousarray(np.asarray(Wh2h, np.float32).astype(f16)),
        "bh2hT": np.ascontiguousarray(
            np.asarray(bh2h, np.float32).reshape(H, 1)),
        "wsc": np.ascontiguousarray(np.tile(
            np.asarray(w_score, np.float32).reshape(H, 1), (1, 128)).astype(f16)),
        "wxc": np.ascontiguousarray(Wx[:D, :].astype(f16)),
        "wxo": np.ascontiguousarray(wxo_p),
        "wh": np.ascontiguousarray(Wh_p.astype(f16)),
        "wg": np.ascontiguousarray(np.asarray(Wg, np.float32).astype(f16)),
    }

    in_maps = []
    for c in range(NCORES):
        bsl = slice(c * BC, (c + 1) * BC)
        in_maps.append({
            "h_nat": np.ascontiguousarray(bh16[bsl]),
            "h_t": np.ascontiguousarray(bh16[bsl].transpose(2, 0, 1)),
            "onehot": np.ascontiguousarray(
                oh[bsl].transpose(2, 1, 0).astype(f16)),
            **weights,
        })
    return in_maps


def kernel(**inputs):
    in_maps = build_in_maps(**inputs)
    bg = inputs["bg"]

    nc = _get_module()
    res = run_bass_kernel_spmd(nc, in_maps, list(range(NCORES)))

    out = np.empty((B, S, C), np.float32)
    for c in range(NCORES):
        out[c * BC:(c + 1) * BC] = res.results[c]["probsT"].transpose(2, 1, 0)
    out += np.asarray(bg, np.float32)[None, None, :]
    return out


if __name__ == "__main__":
    _build()
    print("build OK")
